# revision 49
# baseline (speedup 1.0000x reference)
"""Trainium2 Bass kernel for nn_AttentionModel_63737314672806.

Sharding: data-parallel over batch (B=128) across 8 NeuronCores; each core
processes 16 batch elements (2048 tokens) through the full model. Weights are
replicated (broadcast) to every core. No collectives.

Layout: activations are feature-major ("transposed"):
  xT[p, c, t] = x[token t, feature c*128+p]
so dense layers are psum = matmul(lhsT=W[kc, dout], rhs=xT[kc, tok]) with the
output feature-major again. All heavy matmuls (QKVGO projections and the FFN)
run in fp8 e4m3 with perf_mode=DoubleRow (2 fp8 MACs/cell/cycle): weights are
host-quantized with static power-of-2 scales, activations get fp8 shadow
copies on device, and descales fold into the existing post-PSUM vector ops.
Attention itself stays in fp8/bf16 normal-mode matmuls with the
host-precomputed exp(bias) table trick; softmax runs 4 heads at a time in
[128,512] tiles. LayerNorm statistics are computed full-width ([128,512]
ones-matmuls) so all row math runs on 128 vector lanes and no PE rank-1
broadcasts are needed. The residual streams (xT, h) stay bf16 in SBUF for
accuracy; there is no DRAM round-trip.

PE density: attention's latency chains are interleaved at emission time with
the next group's Q/K/V projection matmuls (and with the gate projection for
the last group) so the in-order PE queue always has dense work and the HAM
clock gate stays warm. Head pooling is done incrementally per FFN sub-block.
"""

import math

import numpy as np
import ml_dtypes

import concourse.bass as bass
import concourse.bacc as bacc
import concourse.mybir as mybir
import concourse.tile as tile
from concourse.bass_utils import run_bass_kernel_spmd

BF16 = mybir.dt.bfloat16
F32 = mybir.dt.float32
FP8 = mybir.dt.float8e4
AF = mybir.ActivationFunctionType
OP = mybir.AluOpType
DR = mybir.MatmulPerfMode.DoubleRow

NCORES = 8
B = 128
L = 128
DFEAT = 32
H = 8
DK = 128
D = 1024
FF = 4096
NL = 2
MAXPOS = 128
OTHER = 64
EPS = 1e-6

BPC = B // NCORES       # 16 batches per core
NTOK = BPC * L          # 2048 tokens per core
NG = 4                  # batch groups per core
GB = BPC // NG          # batches per group = 4
GT = GB * L             # tokens per group = 512
DC = D // 128           # 8 feature chunks
DC2 = DC // 2           # 4 DoubleRow chunk-pairs
FC = FF // 128          # 32 ff chunks
FC2 = FC // 2           # 16 DoubleRow pairs
QSCALE = 1.0 / math.sqrt(float(DK))
SURVIVE = [1.0, 0.5]

# static fp8 scales (powers of two; see docstring)
WS = 2.0 ** 7           # weight scale: wq,wk,wv,wg,wo,wfg,wf2
WS_F1 = 2.0 ** 2        # wf1 (low so f8 = ps1*sig stays < 240)
XS = 2.0 ** 3           # x / h fp8 shadow scale
QS = 2.0 ** 6           # q8/k8 scale (on top of folded QSCALE for q)
VS = 2.0 ** 4           # v8 / ao8 scale

_cached = {}


def _build_nc():
    nc = bacc.Bacc("TRN2", target_bir_lowering=False, debug=False,
                   num_devices=NCORES)

    def din(name, shape, dtype):
        return nc.dram_tensor(name, list(shape), dtype, kind="ExternalInput")

    t = {}
    t["cgmT"] = din("cgmT", [DFEAT, NTOK], BF16)
    t["w_in"] = din("w_in", [DFEAT, D], BF16)
    t["b_in_c"] = din("b_in_c", [128, DC], F32)
    t["b_in8_c"] = din("b_in8_c", [128, DC], F32)
    for w in ("wq8", "wk8", "wv8", "wg8", "wo8"):
        t[w] = din(w, [NL, 128, DC, D], FP8)
    t["wf18"] = din("wf18", [NL, 128, DC, FF], FP8)
    t["wfg8"] = din("wfg8", [NL, 128, DC, FF], FP8)
    t["wf28"] = din("wf28", [NL, 128, FC, D], FP8)
    for bn in ("bq_c", "bk_c", "bg_c", "bo_c", "bf2_c",
               "ln1_s_c", "ln1_ns_c", "ln1_b_c", "ln1_b8_c",
               "ln2_s_c", "ln2_ns_c", "bx_c", "bx8_c"):
        t[bn] = din(bn, [128, NL, DC], F32)
    t["bxp_c"] = din("bxp_c", [128, DC], F32)
    t["bf1_c"] = din("bf1_c", [128, NL, FC], F32)
    t["bfg_c"] = din("bfg_c", [128, NL, FC], F32)
    t["bv_bc"] = din("bv_bc", [128, NL, D], FP8)
    t["exptab"] = din("exptab", [128, NL, 143], BF16)
    t["wd1"] = din("wd1", [128, 17, 128], BF16)
    t["bd1_c"] = din("bd1_c", [128, 1], F32)
    t["ln3_s_c"] = din("ln3_s_c", [128, 1], F32)
    t["ln3_ns_c"] = din("ln3_ns_c", [128, 1], F32)
    t["ln3_b_c"] = din("ln3_b_c", [128, 1], F32)
    t["wd2"] = din("wd2", [128, 128], BF16)
    t["bd2_c"] = din("bd2_c", [128, 1], F32)
    t["wout"] = din("wout", [128, 1], BF16)
    t["bout_t"] = din("bout_t", [1, 1], F32)
    t["otherT"] = din("otherT", [128, BPC], BF16)
    y_out = nc.dram_tensor("y", [1, BPC], F32, kind="ExternalOutput")

    with tile.TileContext(nc, pool_alloc_mode="queue") as tc:
        _emit(nc, tc, t, y_out)
    nc.compile()
    return nc


class Ctx:
    pass


def _emit(nc, tc, t, y_out):
    with (
        tc.tile_pool(name="persist", bufs=1) as pp,
        tc.tile_pool(name="xq", bufs=4) as xqp,
        tc.tile_pool(name="mm_psum", bufs=4, space="PSUM") as mmp,
        tc.tile_pool(name="at_psum", bufs=2, space="PSUM") as app,
        tc.tile_pool(name="st_psum", bufs=1, space="PSUM") as stp,
        tc.tile_pool(name="lnp", bufs=2) as lnp,
        tc.tile_pool(name="resp", bufs=1) as rp,
        tc.tile_pool(name="sqp", bufs=2) as sqp,
    ):
        c = Ctx()
        c.t = t
        c.mmp, c.app, c.stp = mmp, app, stp
        c.lnp, c.rp, c.sqp, c.xqp = lnp, rp, sqp, xqp
        c.pool_pending = []

        # persistent state
        c.xT = pp.tile([128, DC, NTOK], BF16, name="xT")
        c.h = pp.tile([128, DC, NTOK], BF16, name="hT")
        c.h8 = pp.tile([128, DC, NTOK], FP8, name="h8T")
        c.ones_col_bf = pp.tile([128, 1], BF16, name="ones_col")
        nc.vector.memset(c.ones_col_bf, 1.0)
        c.ones128 = pp.tile([128, 128], BF16, name="ones128")
        nc.vector.memset(c.ones128, 1.0)
        c.ones128r = pp.tile([128, 128], mybir.dt.float32r, name="ones128r")
        nc.vector.memset(c.ones128r.bitcast(F32), 1.0)
        c.eps_col = pp.tile([128, 1], F32, name="eps_col")
        nc.vector.memset(c.eps_col, EPS)

        # small constants
        consts = {}
        for name in ("b_in_c", "b_in8_c", "bq_c", "bk_c", "bg_c", "bo_c",
                     "bf2_c", "ln1_s_c", "ln1_ns_c", "ln1_b_c", "ln1_b8_c",
                     "ln2_s_c", "ln2_ns_c", "bx_c", "bx8_c", "bxp_c",
                     "bf1_c", "bfg_c",
                     "bv_bc", "exptab", "bd1_c", "ln3_s_c", "ln3_ns_c",
                     "ln3_b_c", "wd2", "bd2_c", "wout", "bout_t", "otherT"):
            ap = t[name]
            tl = pp.tile(list(ap.shape), ap.dtype, name=f"c_{name}")
            nc.gpsimd.dma_start(out=tl[:], in_=ap[:])
            consts[name] = tl
        c.consts = consts

        with tc.tile_pool(name="wlayer", bufs=1) as wp:
            c.wp = wp
            # layer-0 big weights start streaming before the input proj
            w5 = _load_layer_weights(nc, c, 0)

            # ---- input projection: xT / xq8 ----
            c.xq = [None] * NG
            with tc.tile_pool(name="inp", bufs=1) as inp:
                cgmT_s = inp.tile([DFEAT, NTOK], BF16, name="cgm_s")
                nc.sync.dma_start(out=cgmT_s[:], in_=t["cgmT"][:])
                w_in_s = inp.tile([DFEAT, D], BF16, name="w_in_s")
                nc.sync.dma_start(out=w_in_s[:], in_=t["w_in"][:])
                for g in range(NG):
                    tok = slice(g * GT, (g + 1) * GT)
                    xq_g = xqp.tile([128, DC, GT], FP8, tag="xq", name="xq_g")
                    for dd in range(DC):
                        ps = mmp.tile([128, GT], F32, tag="mm", name="ps_in")
                        nc.tensor.matmul(
                            ps[:], w_in_s[:, dd * 128:(dd + 1) * 128],
                            cgmT_s[:, tok], start=True, stop=True)
                        nc.vector.tensor_scalar_add(
                            out=c.xT[:, dd, tok], in0=ps[:],
                            scalar1=consts["b_in_c"][:, dd:dd + 1])
                        nc.scalar.activation(
                            out=xq_g[:, dd, :], in_=ps[:], func=AF.Identity,
                            bias=consts["b_in8_c"][:, dd:dd + 1], scale=XS)
                    c.xq[g] = xq_g

            # ---- transformer layers ----
            for i in range(NL):
                w5_next = _layer(nc, tc, c, i, w5)
                w5 = w5_next

        # ---- head ----
        _head(nc, tc, c, y_out)


def _load_layer_weights(nc, c, i):
    """Preload the V weight whole-layer; Q/K/O/G stream in per-chunk."""
    w5 = {}
    for nm in ("wv8",):
        tl = c.wp.tile([128, DC, D], FP8, tag=nm, name=f"{nm}_s")
        nc.sync.dma_start(out=tl[:], in_=c.t[nm][i])
        w5[nm] = tl
    return w5


_CHUNK_BUFS = {"wq8": 6, "wk8": 6, "wg8": 4, "wo8": 4}


def _w_chunk(nc, c, nm, i, dd, queue="sync"):
    ch = c.wp.tile([128, DC, 128], FP8, tag=f"{nm}_ch", bufs=_CHUNK_BUFS[nm],
                   name=f"{nm}_ch")
    eng = nc.sync if queue == "sync" else nc.gpsimd
    eng.dma_start(out=ch[:], in_=c.t[nm][i, :, :, dd * 128:(dd + 1) * 128])
    return ch


def _layer(nc, tc, c, i, w5):
    with (
        tc.tile_pool(name="grp", bufs=2) as gp,
        tc.tile_pool(name="attw", bufs=2) as at,
    ):
        c.gp, c.at = gp, at
        qkv = [None, None]   # rotating (q8, k8, v8) per group parity
        qkv[0] = _emit_qkv(nc, c, i, w5, 0, fillers=None)

        for g in range(NG):
            fillers = []
            if g + 1 < NG:
                qkv[(g + 1) % 2] = _emit_qkv(nc, c, i, w5, g + 1,
                                             fillers=fillers)
            # the gate projection only needs xq — it is pure filler material
            sig_t = _emit_gate(nc, c, i, w5, g, fillers=fillers)
            ao8 = _attention(nc, c, i, g, qkv[g % 2], fillers)
            _og_ln1(nc, c, i, w5, g, ao8, qkv[g % 2], sig_pre=sig_t)

    # layer i+1 weights stream during this layer's FFN
    w5_next = _load_layer_weights(nc, c, i + 1) if i + 1 < NL else None

    _ffn(nc, tc, c, i)
    return w5_next


def _emit_qkv(nc, c, i, w5, g, fillers):
    """Q/K/V projections for group g. If fillers is a list, append one
    closure per PSUM-group instead of emitting directly."""
    consts = c.consts
    tok = slice(g * GT, (g + 1) * GT)
    q8 = c.gp.tile([128, H, GT], FP8, tag="q8", name="q8")
    k8 = c.gp.tile([128, H, GT], FP8, tag="k8", name="k8")
    v8 = c.gp.tile([128, GB, D], FP8, tag="v8", name="v8")
    xq = c.xq[g]

    def q_chunk(dd):
        def f():
            wq_ch = _w_chunk(nc, c, "wq8", i, dd)
            ps = c.mmp.tile([128, GT], F32, tag="mm", name="psq")
            for kc in range(DC2):
                nc.tensor.matmul(
                    ps[:], wq_ch[:, 2 * kc:2 * kc + 2, :],
                    xq[:, 2 * kc:2 * kc + 2, :],
                    start=(kc == 0), stop=(kc == DC2 - 1), perf_mode=DR)
            nc.vector.tensor_scalar(
                out=q8[:, dd, :], in0=ps[:],
                scalar1=QS * QSCALE / (WS * XS),
                scalar2=consts["bq_c"][:, i, dd:dd + 1],
                op0=OP.mult, op1=OP.add)
        return f

    def k_chunk(dd):
        def f():
            wk_ch = _w_chunk(nc, c, "wk8", i, dd)
            ps = c.mmp.tile([128, GT], F32, tag="mm", name="psk")
            for kc in range(DC2):
                nc.tensor.matmul(
                    ps[:], wk_ch[:, 2 * kc:2 * kc + 2, :],
                    xq[:, 2 * kc:2 * kc + 2, :],
                    start=(kc == 0), stop=(kc == DC2 - 1), perf_mode=DR)
            nc.vector.tensor_scalar(
                out=k8[:, dd, :], in0=ps[:],
                scalar1=QS / (WS * XS),
                scalar2=consts["bk_c"][:, i, dd:dd + 1],
                op0=OP.mult, op1=OP.add)
        return f

    def v_chunk(cc, jj):
        def f():
            ps = c.mmp.tile([128, 512], F32, tag="mm", name="psv")
            for kc in range(DC2):
                nc.tensor.matmul(
                    ps[:], xq[:, 2 * kc:2 * kc + 2, jj * L:(jj + 1) * L],
                    w5["wv8"][:, 2 * kc:2 * kc + 2,
                              cc * 512:(cc + 1) * 512],
                    start=(kc == 0), stop=(kc == DC2 - 1), perf_mode=DR)
            nc.vector.scalar_tensor_tensor(
                out=v8[:, jj, cc * 512:(cc + 1) * 512], in0=ps[:],
                scalar=VS / (WS * XS),
                in1=consts["bv_bc"][:, i, cc * 512:(cc + 1) * 512],
                op0=OP.mult, op1=OP.add)
        return f

    closures = ([q_chunk(dd) for dd in range(DC)]
                + [k_chunk(dd) for dd in range(DC)]
                + [v_chunk(cc, jj) for cc in range(2) for jj in range(GB)])
    if fillers is None:
        for f in closures:
            f()
    else:
        fillers.extend(closures)
    return q8, k8, v8


def _emit_gate(nc, c, i, w5, g, fillers):
    """Gate projection sigmoid(x@Wg+bg) for group g (filler closures)."""
    consts = c.consts
    xq = c.xq[g]
    sig_t = c.rp.tile([128, DC, GT], BF16, tag="sig", name="sig_g")

    def g_chunk(dd):
        def f():
            wg_ch = _w_chunk(nc, c, "wg8", i, dd, queue="gpsimd")
            ps = c.mmp.tile([128, GT], F32, tag="mm", name="psg")
            for kc in range(DC2):
                nc.tensor.matmul(
                    ps[:], wg_ch[:, 2 * kc:2 * kc + 2, :],
                    xq[:, 2 * kc:2 * kc + 2, :],
                    start=(kc == 0), stop=(kc == DC2 - 1), perf_mode=DR)
            nc.scalar.activation(
                out=sig_t[:, dd, :], in_=ps[:], func=AF.Sigmoid,
                bias=consts["bg_c"][:, i, dd:dd + 1], scale=1.0 / (WS * XS))
        return f

    fillers.extend(g_chunk(dd) for dd in range(DC))
    return sig_t


def _attention(nc, c, i, g, qkv, fillers):
    """Attention for group g, one wave per batch (8 heads in 2 half-waves).
    Emits filler closures between dependency steps to keep the PE dense."""
    q8, k8, v8 = qkv
    fi = iter(fillers)

    def pump(n):
        for _ in range(n):
            f = next(fi, None)
            if f is None:
                return
            f()

    ao8 = c.gp.tile([128, H, GB, L], FP8, tag="ao8", bufs=1, name="ao8")
    for jj in range(GB):
        b_local = g * GB + jj
        jtok = slice(jj * L, (jj + 1) * L)
        etab = c.consts["exptab"][:, i, 15 - b_local:143 - b_local]
        pa = [None, None]
        awe2 = [None, None]
        for half in range(2):
            h0 = half * 4
            pa[half] = c.app.tile([128, 512], F32, tag="pa", name="pa")
            for hh in range(4):
                nc.tensor.matmul(
                    pa[half][:, hh * L:(hh + 1) * L],
                    k8[:, h0 + hh, jtok], q8[:, h0 + hh, jtok],
                    start=True, stop=True)
        pump(3)
        rb = [None, None]
        for half in range(2):
            awe = c.at.tile([128, 512], BF16, tag="awe", name="awe")
            nc.scalar.activation(out=awe[:], in_=pa[half][:], func=AF.Exp,
                                 scale=1.0 / (QS * QS))
            # in-place multiply by the per-batch exp(bias) table
            nc.vector.tensor_tensor(
                out=awe[:].rearrange("p (h q) -> p h q", q=L),
                in0=awe[:].rearrange("p (h q) -> p h q", q=L),
                in1=etab.unsqueeze(1).to_broadcast([128, 4, L]), op=OP.mult)
            awe2[half] = awe
        pump(2)
        for half in range(2):
            # softmax sums land in partition 0 of the (already-read) logit
            # psum tile — saves a PSUM bank
            sm = pa[half][0:1, :]
            nc.tensor.matmul(sm, c.ones_col_bf[:, 0:1], awe2[half][:],
                             start=True, stop=True)
            smr = c.at.tile([1, 512], F32, tag="smr", bufs=1, name="smr")
            nc.scalar.copy(out=smr[:], in_=sm)
            sb = c.lnp.tile([128, 512], F32, tag="rs", name="sb")
            nc.gpsimd.partition_broadcast(sb[:], smr[:])
            rb[half] = c.lnp.tile([128, 512], F32, tag="mrs", name="rb")
            nc.vector.reciprocal_approx_fast(out=rb[half][:], in_=sb[:])
        pump(3)
        for half in range(2):
            h0 = half * 4
            pao = c.app.tile([128, 512], F32, tag="pa", name="pao")
            for hh in range(4):
                nc.tensor.matmul(
                    pao[:, hh * L:(hh + 1) * L],
                    v8[:, jj, (h0 + hh) * DK:(h0 + hh + 1) * DK],
                    awe2[half][:, hh * L:(hh + 1) * L], start=True, stop=True)
            nc.vector.tensor_tensor(
                out=ao8[:, h0:h0 + 4, jj, :],
                in0=pao[:].rearrange("p (h q) -> p h q", q=L),
                in1=rb[half][:].rearrange("p (h q) -> p h q", q=L),
                op=OP.mult)
    # drain any remaining fillers
    pump(1 << 30)
    return ao8


def _og_ln1(nc, c, i, w5, g, ao8, qkv, sig_pre):
    """Gate + O-projection + residual + LN1 for group g."""
    consts = c.consts
    tok = slice(g * GT, (g + 1) * GT)
    xq = c.xq[g]

    sig_t = sig_pre

    res_t = c.rp.tile([128, DC, GT], BF16, tag="res", name="res_t")
    ps_s = c.stp.tile([128, GT], F32, tag="ss", name="ps_s")
    ps_q = c.stp.tile([128, GT], F32, tag="sq", name="ps_q")
    for dd in range(DC):
        wo_ch = _w_chunk(nc, c, "wo8", i, dd, queue="gpsimd")
        pso = c.mmp.tile([128, GT], F32, tag="mm", name="pso")
        for kc in range(DC2):
            nc.tensor.matmul(
                pso[:], wo_ch[:, 2 * kc:2 * kc + 2, :],
                ao8[:, 2 * kc:2 * kc + 2, :, :],
                start=(kc == 0), stop=(kc == DC2 - 1), perf_mode=DR)
        t1 = c.sqp.tile([128, GT], F32, tag="t1", bufs=1, name="t1")
        nc.vector.tensor_scalar(
            out=t1[:], in0=pso[:], scalar1=1.0 / (WS * VS),
            scalar2=consts["bo_c"][:, i, dd:dd + 1], op0=OP.mult, op1=OP.add)
        nc.vector.tensor_mul(out=t1[:], in0=t1[:], in1=sig_t[:, dd, :])
        # res = (xT_nob + bx) + gated-attn; bx folds the previous layer's
        # deferred LN2 bias (zero for layer 0, whose xT carries b_in)
        nc.vector.scalar_tensor_tensor(
            out=res_t[:, dd, :], in0=c.xT[:, dd, tok],
            scalar=consts["bx_c"][:, i, dd:dd + 1], in1=t1[:],
            op0=OP.add, op1=OP.add)
        sq = c.sqp.tile([128, GT], BF16, tag="sq", name="sq")
        nc.scalar.activation(out=sq[:], in_=res_t[:, dd, :], func=AF.Square)
        nc.tensor.matmul(ps_s[:], c.ones128[:], res_t[:, dd, :],
                         start=(dd == 0), stop=(dd == DC - 1))
        nc.tensor.matmul(ps_q[:], c.ones128[:], sq[:],
                         start=(dd == 0), stop=(dd == DC - 1))

    rs_f, mrs_f = _ln_rows_full(nc, c, ps_s, ps_q, GT, 1.0 / D)
    for dd in range(DC):
        u = c.lnp.tile([128, GT], F32, tag="u", bufs=1, name="u")
        nc.vector.scalar_tensor_tensor(
            out=u[:], in0=res_t[:, dd, :],
            scalar=consts["ln1_s_c"][:, i, dd:dd + 1], in1=rs_f[:],
            op0=OP.mult, op1=OP.mult)
        # h is stored WITHOUT ln1_b; the bias is folded into the two
        # consumers (h8 shadow below, f2-residual add in _ffn)
        nc.vector.scalar_tensor_tensor(
            out=c.h[:, dd, tok], in0=mrs_f[:],
            scalar=consts["ln1_ns_c"][:, i, dd:dd + 1], in1=u[:],
            op0=OP.mult, op1=OP.add)
        nc.scalar.activation(out=c.h8[:, dd, tok], in_=c.h[:, dd, tok],
                             func=AF.Identity,
                             bias=consts["ln1_b8_c"][:, i, dd:dd + 1],
                             scale=XS)


def _ln_rows_full(nc, c, ps_s, ps_q, n, inv_d):
    """Full-width LN stats: rs = 1/sqrt(var+eps), mrs = mean*rs as
    [128, n] tiles (all rows identical)."""
    m_f = c.lnp.tile([128, 512], F32, tag="m", bufs=1, name="m_f")[:, :n]
    nc.vector.tensor_scalar_mul(out=m_f, in0=ps_s[:], scalar1=inv_d)
    m2 = c.lnp.tile([128, 512], F32, tag="tmp", bufs=2, name="m2")[:, :n]
    nc.vector.tensor_mul(out=m2, in0=m_f, in1=m_f)
    var = c.lnp.tile([128, 512], F32, tag="tmp", bufs=2, name="var")[:, :n]
    nc.vector.scalar_tensor_tensor(out=var, in0=ps_q[:], scalar=inv_d,
                                   in1=m2, op0=OP.mult, op1=OP.subtract)
    std = c.lnp.tile([128, 512], F32, tag="tmp", bufs=2, name="std")[:, :n]
    nc.scalar.activation(out=std, in_=var, func=AF.Sqrt, bias=c.eps_col[:],
                         scale=1.0)
    rs_f = c.lnp.tile([128, 512], F32, tag="rs", name="rs_f")[:, :n]
    nc.vector.reciprocal_approx_fast(out=rs_f, in_=std)
    mrs_f = c.lnp.tile([128, 512], F32, tag="mrs", name="mrs_f")[:, :n]
    nc.vector.tensor_mul(out=mrs_f, in0=m_f, in1=rs_f)
    return rs_f, mrs_f


def _ffn(nc, tc, c, i):
    consts = c.consts
    with (
        tc.tile_pool(name="fbuf", bufs=1) as fp,
        tc.tile_pool(name="fwch", bufs=4) as wc,
        tc.tile_pool(name="fw2ch", bufs=2) as wc2,
    ):
        for sub in range(NG):
            tok = slice(sub * GT, (sub + 1) * GT)
            f8 = fp.tile([128, FC, GT], FP8, tag="f8", name="f8")
            # --- f8 = (h@Wf1 + bf1) * sigmoid(h@Wfg + bfg), fp8-scaled ---
            for fc in range(FC):
                if fc % 8 == 4 and c.pool_pending:
                    c.pool_pending.pop(0)()
                wf1_ch = wc.tile([128, DC, 128], FP8, tag="wf1", bufs=4, name="wf1c")
                nc.sync.dma_start(
                    out=wf1_ch[:],
                    in_=c.t["wf18"][i, :, :, fc * 128:(fc + 1) * 128])
                wfg_ch = wc.tile([128, DC, 128], FP8, tag="wfg", bufs=4, name="wfgc")
                nc.gpsimd.dma_start(
                    out=wfg_ch[:],
                    in_=c.t["wfg8"][i, :, :, fc * 128:(fc + 1) * 128])
                ps1 = c.mmp.tile([128, GT], F32, tag="mm", name="ps1")
                psg = c.mmp.tile([128, GT], F32, tag="mm", name="psfg")
                for kc in range(DC2):
                    nc.tensor.matmul(
                        ps1[:], wf1_ch[:, 2 * kc:2 * kc + 2, :],
                        c.h8[:, 2 * kc:2 * kc + 2, tok],
                        start=(kc == 0), stop=(kc == DC2 - 1), perf_mode=DR)
                for kc in range(DC2):
                    nc.tensor.matmul(
                        psg[:], wfg_ch[:, 2 * kc:2 * kc + 2, :],
                        c.h8[:, 2 * kc:2 * kc + 2, tok],
                        start=(kc == 0), stop=(kc == DC2 - 1), perf_mode=DR)
                sig = c.sqp.tile([128, GT], BF16, tag="fsig", name="fsig")
                nc.scalar.activation(
                    out=sig[:], in_=psg[:], func=AF.Sigmoid,
                    bias=consts["bfg_c"][:, i, fc:fc + 1],
                    scale=1.0 / (WS * XS))
                nc.vector.scalar_tensor_tensor(
                    out=f8[:, fc, :], in0=ps1[:],
                    scalar=consts["bf1_c"][:, i, fc:fc + 1], in1=sig[:],
                    op0=OP.add, op1=OP.mult)

            # --- f @ Wf2 + bf2 + h, LN2, stochastic-depth blend into xT ---
            res_t = c.rp.tile([128, DC, GT], BF16, tag="res", name="res2")
            ps_s = c.stp.tile([128, GT], F32, tag="ss", name="ps_s2")
            ps_q = c.stp.tile([128, GT], F32, tag="sq", name="ps_q2")
            for dd in range(DC):
                wf2_ch = wc2.tile([128, FC, 128], FP8, tag="wf2", name="wf2c")
                nc.gpsimd.dma_start(
                    out=wf2_ch[:],
                    in_=c.t["wf28"][i, :, :, dd * 128:(dd + 1) * 128])
                ps2 = c.mmp.tile([128, GT], F32, tag="mm", name="ps2")
                for fc in range(FC2):
                    nc.tensor.matmul(
                        ps2[:], wf2_ch[:, 2 * fc:2 * fc + 2, :],
                        f8[:, 2 * fc:2 * fc + 2, :],
                        start=(fc == 0), stop=(fc == FC2 - 1), perf_mode=DR)
                t1 = c.sqp.tile([128, GT], F32, tag="t1", bufs=1, name="ft1")
                nc.vector.tensor_scalar(
                    out=t1[:], in0=ps2[:], scalar1=1.0 / (WS * WS_F1 * XS),
                    scalar2=consts["bf2_c"][:, i, dd:dd + 1],
                    op0=OP.mult, op1=OP.add)
                # h is stored without ln1_b; add it back here
                nc.vector.scalar_tensor_tensor(
                    out=res_t[:, dd, :], in0=c.h[:, dd, tok],
                    scalar=consts["ln1_b_c"][:, i, dd:dd + 1], in1=t1[:],
                    op0=OP.add, op1=OP.add)
                sq = c.sqp.tile([128, GT], BF16, tag="sq", name="fsq")
                nc.scalar.activation(out=sq[:], in_=res_t[:, dd, :],
                                     func=AF.Square)
                nc.tensor.matmul(ps_s[:], c.ones128[:], res_t[:, dd, :],
                                 start=(dd == 0), stop=(dd == DC - 1))
                nc.tensor.matmul(ps_q[:], c.ones128[:], sq[:],
                                 start=(dd == 0), stop=(dd == DC - 1))

            rs_f, mrs_f = _ln_rows_full(nc, c, ps_s, ps_q, GT, 1.0 / D)
            sv = SURVIVE[i]
            for dd in range(DC):
                u = c.lnp.tile([128, GT], F32, tag="u", bufs=1, name="fu")
                nc.vector.scalar_tensor_tensor(
                    out=u[:], in0=res_t[:, dd, :],
                    scalar=consts["ln2_s_c"][:, i, dd:dd + 1], in1=rs_f[:],
                    op0=OP.mult, op1=OP.mult)
                if sv == 1.0:
                    nc.vector.scalar_tensor_tensor(
                        out=c.xT[:, dd, tok], in0=mrs_f[:],
                        scalar=consts["ln2_ns_c"][:, i, dd:dd + 1], in1=u[:],
                        op0=OP.mult, op1=OP.add)
                else:
                    v1 = c.lnp.tile([128, GT], F32, tag="v1", bufs=1,
                                    name="fv1")
                    nc.vector.scalar_tensor_tensor(
                        out=v1[:], in0=mrs_f[:],
                        scalar=consts["ln2_ns_c"][:, i, dd:dd + 1], in1=u[:],
                        op0=OP.mult, op1=OP.add)
                    nc.vector.scalar_tensor_tensor(
                        out=c.xT[:, dd, tok], in0=c.xT[:, dd, tok],
                        scalar=1.0 - sv, in1=v1[:], op0=OP.mult, op1=OP.add)
                # xT is stored WITHOUT the (survive-scaled) ln2 bias; it is
                # folded into the consumers (xq shadow, next layer's
                # O-residual via bx_c, pooling via bxp_c)
                if i + 1 < NL:
                    # fp8 shadow for the next layer's projections
                    if dd == 0:
                        c.xq[sub] = c.xqp.tile([128, DC, GT], FP8, tag="xq",
                                               name="xq_n")
                    nc.scalar.activation(out=c.xq[sub][:, dd, :],
                                         in_=c.xT[:, dd, tok],
                                         func=AF.Identity,
                                         bias=consts["bx8_c"][:, i + 1,
                                                             dd:dd + 1],
                                         scale=XS)
            if i + 1 == NL:
                # head pooling: enqueue one small reduce per batch; they are
                # drained inside the next sub's fc loop so the 1.2us vector
                # ops never block the PE's psum drains in a burst
                if sub == 0:
                    c.pool_sum = c.lnp.tile([128, DC, BPC], F32, tag="pls",
                                            bufs=1, name="pool_sum")
                    c.pool_max = c.lnp.tile([128, DC, BPC], F32, tag="plm",
                                            bufs=1, name="pool_max")

                def mk_pool(b_abs):
                    def f():
                        xv = c.xT[:, :, b_abs * L:(b_abs + 1) * L]
                        nc.vector.tensor_reduce(
                            out=c.pool_sum[:, :, b_abs], in_=xv,
                            axis=mybir.AxisListType.X, op=OP.add)
                        nc.vector.tensor_reduce(
                            out=c.pool_max[:, :, b_abs], in_=xv,
                            axis=mybir.AxisListType.X, op=OP.max)
                    return f
                for jj in range(GB):
                    c.pool_pending.append(mk_pool(sub * GB + jj))


def _head(nc, tc, c, y_out):
    consts = c.consts
    with tc.tile_pool(name="head", bufs=1) as hp:
        for f in c.pool_pending:
            f()
        c.pool_pending = []
        wd1_s = hp.tile([128, 17, 128], BF16, name="wd1_s")
        nc.sync.dma_start(out=wd1_s[:], in_=c.t["wd1"][:])
        poolT = hp.tile([128, 17, BPC], BF16, name="poolT")
        for dd in range(DC):
            # mean/max pooling of xT_nob + deferred final LN2 bias (bxp)
            nc.vector.tensor_scalar(
                out=poolT[:, dd, :], in0=c.pool_sum[:, dd, :],
                scalar1=1.0 / L, scalar2=consts["bxp_c"][:, dd:dd + 1],
                op0=OP.mult, op1=OP.add)
            nc.vector.tensor_scalar_add(
                out=poolT[:, DC + dd, :], in0=c.pool_max[:, dd, :],
                scalar1=consts["bxp_c"][:, dd:dd + 1])
        nc.gpsimd.tensor_copy(out=poolT[:, 16, :], in_=consts["otherT"][:])

        # y1 = relu(pooled @ Wd1 + bd1)   [128 dout, 16]
        ps1 = c.mmp.tile([128, GT], F32, tag="mm", name="hps")[:, 0:BPC]
        for cc in range(17):
            nc.tensor.matmul(ps1, wd1_s[:, cc, :], poolT[:, cc, :],
                             start=(cc == 0), stop=(cc == 16))
        y1 = hp.tile([128, BPC], mybir.dt.float32r, name="y1")
        with nc.allow_low_precision(reason="fp32r head LN within tolerance"):
            nc.scalar.activation(out=y1[:], in_=ps1, func=AF.Relu,
                                 bias=consts["bd1_c"][:], scale=1.0)

        # LN3 over the 128 features (partition dim), full-width stats
        sq3 = hp.tile([128, BPC], mybir.dt.float32r, name="sq3")
        with nc.allow_low_precision(reason="fp32r head LN within tolerance"):
            nc.vector.tensor_mul(out=sq3[:], in0=y1[:].bitcast(F32),
                                 in1=y1[:].bitcast(F32))
        ps_s = c.stp.tile([128, GT], F32, tag="ss", name="hs")[:, 0:BPC]
        ps_q = c.stp.tile([128, GT], F32, tag="sq", name="hq")[:, 0:BPC]
        nc.tensor.matmul(ps_s, c.ones128r[:], y1[:], start=True, stop=True)
        nc.tensor.matmul(ps_q, c.ones128r[:], sq3[:], start=True, stop=True)
        rs_f, mrs_f = _ln_rows_full(nc, c, ps_s, ps_q, BPC, 1.0 / 128)
        u3 = hp.tile([128, BPC], F32, name="u3")
        nc.vector.scalar_tensor_tensor(
            out=u3[:], in0=y1[:].bitcast(F32), scalar=consts["ln3_s_c"][:],
            in1=rs_f[:], op0=OP.mult, op1=OP.mult)
        yln0 = hp.tile([128, BPC], F32, name="yln0")
        nc.vector.scalar_tensor_tensor(
            out=yln0[:], in0=mrs_f[:], scalar=consts["ln3_ns_c"][:],
            in1=u3[:], op0=OP.mult, op1=OP.add)
        yln = hp.tile([128, BPC], BF16, name="yln")
        nc.vector.tensor_scalar_add(out=yln[:], in0=yln0[:],
                                    scalar1=consts["ln3_b_c"][:])

        # y2 = relu(yln @ Wd2 + bd2); y = y2 @ Wout + bout
        ps2 = c.mmp.tile([128, GT], F32, tag="mm", name="hps2")[:, 0:BPC]
        nc.tensor.matmul(ps2, consts["wd2"][:], yln[:], start=True, stop=True)
        y2 = hp.tile([128, BPC], BF16, name="y2")
        nc.scalar.activation(out=y2[:], in_=ps2, func=AF.Relu,
                             bias=consts["bd2_c"][:], scale=1.0)
        psy = c.mmp.tile([128, GT], F32, tag="mm", name="hpsy")[0:1, 0:BPC]
        nc.tensor.matmul(psy, consts["wout"][:], y2[:], start=True, stop=True)
        yfin = hp.tile([1, BPC], F32, name="yfin")
        nc.vector.tensor_tensor(
            out=yfin[:], in0=psy,
            in1=consts["bout_t"][:].to_broadcast([1, BPC]), op=OP.add)
        nc.sync.dma_start(out=y_out[:], in_=yfin[:])


# ---------------------------------------------------------------------------
# host side
# ---------------------------------------------------------------------------

def _bf(x):
    return np.ascontiguousarray(np.asarray(x, np.float32)).astype(
        ml_dtypes.bfloat16)


def _f32(x):
    return np.ascontiguousarray(np.asarray(x, np.float32))


def _fp8(x, scale):
    a = np.asarray(x, np.float32) * np.float32(scale)
    a = np.clip(a, -240.0, 240.0)
    return np.ascontiguousarray(a).astype(ml_dtypes.float8_e4m3)


def _prep_shared(I):
    sv = np.array(SURVIVE, np.float32)
    s = {}
    s["w_in"] = _bf(I["W_in"])
    b_in = np.asarray(I["b_in"], np.float32).reshape(DC, 128).T
    s["b_in_c"] = _f32(b_in)
    s["b_in8_c"] = _f32(b_in * XS)
    for nm, W, ws in (("wq8", I["Wq"], WS), ("wk8", I["Wk"], WS),
                      ("wv8", I["Wv"], WS), ("wg8", I["Wg"], WS),
                      ("wo8", I["Wo"], WS)):
        s[nm] = _fp8(np.asarray(W, np.float32).reshape(NL, DC, 128, D)
                     .transpose(0, 2, 1, 3), ws)
    s["wf18"] = _fp8(np.asarray(I["Wf1"], np.float32)
                     .reshape(NL, DC, 128, FF).transpose(0, 2, 1, 3), WS_F1)
    s["wfg8"] = _fp8(np.asarray(I["Wfg"], np.float32)
                     .reshape(NL, DC, 128, FF).transpose(0, 2, 1, 3), WS)
    s["wf28"] = _fp8(np.asarray(I["Wf2"], np.float32)
                     .reshape(NL, FC, 128, D).transpose(0, 2, 1, 3), WS)

    def col(b, nch):
        return _f32(np.asarray(b, np.float32).reshape(NL, nch, 128)
                    .transpose(2, 0, 1))

    s["bq_c"] = col(np.asarray(I["bq"], np.float32) * (QS * QSCALE), DC)
    s["bk_c"] = col(np.asarray(I["bk"], np.float32) * QS, DC)
    s["bg_c"] = col(I["bg"], DC)
    s["bo_c"] = col(I["bo"], DC)
    s["bf2_c"] = col(I["bf2"], DC)
    s["bf1_c"] = col(np.asarray(I["bf1"], np.float32) * (WS_F1 * XS), FC)
    s["bfg_c"] = col(I["bfg"], FC)
    ln1_s = np.asarray(I["ln1_s"], np.float32)
    ln2_s = np.asarray(I["ln2_s"], np.float32) * sv[:, None]
    s["ln1_s_c"] = col(ln1_s, DC)
    s["ln1_ns_c"] = col(-ln1_s, DC)
    s["ln1_b_c"] = col(I["ln1_b"], DC)
    s["ln1_b8_c"] = col(np.asarray(I["ln1_b"], np.float32) * XS, DC)
    s["ln2_s_c"] = col(ln2_s, DC)
    s["ln2_ns_c"] = col(-ln2_s, DC)
    # deferred (survive-scaled) LN2 bias for the residual stream:
    # bx[i] = bias carried by xT entering layer i (0 for layer 0 — its xT
    # includes b_in directly); bxp = bias at pooling time.
    ln2_b = np.asarray(I["ln2_b"], np.float32)
    bx = np.zeros((NL, D), np.float32)
    running = np.zeros(D, np.float32)
    for li in range(NL):
        bx[li] = running
        running = (1.0 - sv[li]) * running + sv[li] * ln2_b[li]
    s["bx_c"] = col(bx, DC)
    s["bx8_c"] = col(bx * XS, DC)
    s["bxp_c"] = _f32(running.reshape(DC, 128).T)
    s["bv_bc"] = _fp8(np.broadcast_to(
        np.asarray(I["bv"], np.float32)[None] * VS, (128, NL, D)), 1.0)
    s["wd1"] = _bf(np.concatenate(
        [np.asarray(I["Wd1"], np.float32),
         np.zeros((17 * 128 - I["Wd1"].shape[0], 128), np.float32)],
        axis=0).reshape(17, 128, 128).transpose(1, 0, 2))
    s["bd1_c"] = _f32(I["bd1"].reshape(128, 1))
    ln3_s = np.asarray(I["ln3_s"], np.float32)
    s["ln3_s_c"] = _f32(ln3_s.reshape(128, 1))
    s["ln3_ns_c"] = _f32(-ln3_s.reshape(128, 1))
    s["ln3_b_c"] = _f32(I["ln3_b"].reshape(128, 1))
    s["wd2"] = _bf(I["Wd2"])
    s["bd2_c"] = _f32(I["bd2"].reshape(128, 1))
    s["wout"] = _bf(I["Wout"])
    s["bout_t"] = _f32(I["bout"].reshape(1, 1))
    return s


def _prep_core(I, shared, cidx):
    m = dict(shared)
    cgm = np.asarray(I["cgm"], np.float32)
    m["cgmT"] = _bf(cgm[cidx * BPC:(cidx + 1) * BPC].reshape(NTOK, DFEAT).T)
    oth = np.asarray(I["other"], np.float32)[cidx * BPC:(cidx + 1) * BPC]
    m["otherT"] = _bf(np.concatenate(
        [oth.T, np.zeros((128 - OTHER, BPC), np.float32)], axis=0))
    # exp of transposed+flipped rel-pos bias table, per-core column window
    rel = np.asarray(I["rel_emb"], np.float32)          # [NL, 255, 128]
    flippedT = rel[:, ::-1, :].transpose(0, 2, 1)       # [NL, 128, 255]
    lo = 112 - 16 * cidx
    tab = flippedT[:, :, lo:lo + 143]                   # [NL, 128, 143]
    m["exptab"] = _bf(np.exp(tab).transpose(1, 0, 2))   # [128, NL, 143]
    return m


def kernel(**inputs) -> np.ndarray:
    if "nc" not in _cached:
        _cached["nc"] = _build_nc()
    nc = _cached["nc"]
    shared = _prep_shared(inputs)
    in_maps = [_prep_core(inputs, shared, cidx) for cidx in range(NCORES)]
    res = run_bass_kernel_spmd(nc, in_maps, core_ids=list(range(NCORES)))
    y = np.concatenate([res.results[cidx]["y"].reshape(BPC)
                        for cidx in range(NCORES)])
    return y.reshape(B, 1).astype(np.float32)


# revision 53
# speedup vs baseline: 1.1173x; 1.1173x over previous
"""Trainium2 Bass kernel for nn_AttentionModel_63737314672806.

Sharding: data-parallel over batch (B=128) across 8 NeuronCores; each core
processes 16 batch elements (2048 tokens) through the full model. Weights are
replicated (broadcast) to every core. No collectives.

Layout: activations are feature-major ("transposed"):
  xT[p, c, t] = x[token t, feature c*128+p]
so dense layers are psum = matmul(lhsT=W[kc, dout], rhs=xT[kc, tok]) with the
output feature-major again. All heavy matmuls (QKVGO projections and the FFN)
run in fp8 e4m3 with perf_mode=DoubleRow (2 fp8 MACs/cell/cycle): weights are
host-quantized with static power-of-2 scales, activations get fp8 shadow
copies on device, and descales fold into the existing post-PSUM vector ops.
Attention itself stays in fp8/bf16 normal-mode matmuls with the
host-precomputed exp(bias) table trick; softmax runs 4 heads at a time in
[128,512] tiles. LayerNorm statistics are computed full-width ([128,512]
ones-matmuls) so all row math runs on 128 vector lanes and no PE rank-1
broadcasts are needed. The residual streams (xT, h) stay bf16 in SBUF for
accuracy; there is no DRAM round-trip.

PE density: attention's latency chains are interleaved at emission time with
the next group's Q/K/V projection matmuls (and with the gate projection for
the last group) so the in-order PE queue always has dense work and the HAM
clock gate stays warm. Head pooling is done incrementally per FFN sub-block.
"""

import math

import numpy as np
import ml_dtypes

import concourse.bass as bass
import concourse.bacc as bacc
import concourse.mybir as mybir
import concourse.tile as tile
from concourse.bass_utils import run_bass_kernel_spmd

BF16 = mybir.dt.bfloat16
F32 = mybir.dt.float32
FP8 = mybir.dt.float8e4
AF = mybir.ActivationFunctionType
OP = mybir.AluOpType
DR = mybir.MatmulPerfMode.DoubleRow

NCORES = 8
B = 128
L = 128
DFEAT = 32
H = 8
DK = 128
D = 1024
FF = 4096
NL = 2
MAXPOS = 128
OTHER = 64
EPS = 1e-6

BPC = B // NCORES       # 16 batches per core
NTOK = BPC * L          # 2048 tokens per core
NG = 4                  # batch groups per core
GB = BPC // NG          # batches per group = 4
GT = GB * L             # tokens per group = 512
DC = D // 128           # 8 feature chunks
DC2 = DC // 2           # 4 DoubleRow chunk-pairs
FC = FF // 128          # 32 ff chunks
FC2 = FC // 2           # 16 DoubleRow pairs
QSCALE = 1.0 / math.sqrt(float(DK))
SURVIVE = [1.0, 0.5]

# static fp8 scales (powers of two; see docstring)
WS = 2.0 ** 7           # weight scale: wq,wk,wv,wg,wo,wfg,wf2
WS_F1 = 2.0 ** 2        # wf1 (low so f8 = ps1*sig stays < 240)
XS = 2.0 ** 3           # x / h fp8 shadow scale
QS = 2.0 ** 6           # q8/k8 scale (on top of folded QSCALE for q)
VS = 2.0 ** 4           # v8 / ao8 scale

_cached = {}


def _build_nc():
    nc = bacc.Bacc("TRN2", target_bir_lowering=False, debug=False,
                   num_devices=NCORES)

    def din(name, shape, dtype):
        return nc.dram_tensor(name, list(shape), dtype, kind="ExternalInput")

    t = {}
    t["cgmT"] = din("cgmT", [DFEAT, NTOK], BF16)
    t["w_in"] = din("w_in", [DFEAT, D], BF16)
    t["b_in_c"] = din("b_in_c", [128, DC], F32)
    t["b_in8_c"] = din("b_in8_c", [128, DC], F32)
    for w in ("wq8", "wk8", "wv8", "wg8", "wo8"):
        t[w] = din(w, [NL, 128, DC, D], FP8)
    t["wf18"] = din("wf18", [NL, 128, DC, FF], FP8)
    t["wfg8"] = din("wfg8", [NL, 128, DC, FF], FP8)
    t["wf28"] = din("wf28", [NL, 128, FC, D], FP8)
    for bn in ("bq_c", "bk_c", "bg_c", "bo_c", "bf2_c",
               "ln1_s_c", "ln1_ns_c", "ln1_b_c", "ln1_b8_c",
               "ln2_s_c", "ln2_ns_c", "bx_c", "bx8_c"):
        t[bn] = din(bn, [128, NL, DC], F32)
    t["bxp_c"] = din("bxp_c", [128, DC], F32)
    t["bf1_c"] = din("bf1_c", [128, NL, FC], F32)
    t["bfg_c"] = din("bfg_c", [128, NL, FC], F32)
    t["bv_bc"] = din("bv_bc", [128, NL, D], FP8)
    t["exptab"] = din("exptab", [128, NL, 143], BF16)
    t["wd1"] = din("wd1", [128, 17, 128], BF16)
    t["bd1_c"] = din("bd1_c", [128, 1], F32)
    t["ln3_s_c"] = din("ln3_s_c", [128, 1], F32)
    t["ln3_ns_c"] = din("ln3_ns_c", [128, 1], F32)
    t["ln3_b_c"] = din("ln3_b_c", [128, 1], F32)
    t["wd2"] = din("wd2", [128, 128], BF16)
    t["bd2_c"] = din("bd2_c", [128, 1], F32)
    t["wout"] = din("wout", [128, 1], BF16)
    t["bout_t"] = din("bout_t", [1, 1], F32)
    t["otherT"] = din("otherT", [128, BPC], BF16)
    y_out = nc.dram_tensor("y", [1, BPC], F32, kind="ExternalOutput")

    with tile.TileContext(nc, pool_alloc_mode="queue") as tc:
        _emit(nc, tc, t, y_out)
    nc.compile()
    return nc


class Ctx:
    pass


def _emit(nc, tc, t, y_out):
    with (
        tc.tile_pool(name="persist", bufs=1) as pp,
        tc.tile_pool(name="xq", bufs=4) as xqp,
        tc.tile_pool(name="mm_psum", bufs=4, space="PSUM") as mmp,
        tc.tile_pool(name="at_psum", bufs=2, space="PSUM") as app,
        tc.tile_pool(name="st_psum", bufs=1, space="PSUM") as stp,
        tc.tile_pool(name="lnp", bufs=2) as lnp,
        tc.tile_pool(name="resp", bufs=1) as rp,
        tc.tile_pool(name="sqp", bufs=2) as sqp,
    ):
        c = Ctx()
        c.t = t
        c.mmp, c.app, c.stp = mmp, app, stp
        c.lnp, c.rp, c.sqp, c.xqp = lnp, rp, sqp, xqp
        c.pool_pending = []

        # persistent state
        c.xT = pp.tile([128, DC, NTOK], BF16, name="xT")
        c.h = pp.tile([128, DC, NTOK], BF16, name="hT")
        c.h8 = pp.tile([128, DC, NTOK], FP8, name="h8T")
        c.ones_col_bf = pp.tile([128, 1], BF16, name="ones_col")
        nc.vector.memset(c.ones_col_bf, 1.0)
        c.ones128 = pp.tile([128, 128], BF16, name="ones128")
        nc.vector.memset(c.ones128, 1.0)
        c.ones128r = pp.tile([128, 128], mybir.dt.float32r, name="ones128r")
        nc.vector.memset(c.ones128r.bitcast(F32), 1.0)
        c.eps_col = pp.tile([128, 1], F32, name="eps_col")
        nc.vector.memset(c.eps_col, EPS)

        # small constants
        consts = {}
        for name in ("b_in_c", "b_in8_c", "bq_c", "bk_c", "bg_c", "bo_c",
                     "bf2_c", "ln1_s_c", "ln1_ns_c", "ln1_b_c", "ln1_b8_c",
                     "ln2_s_c", "ln2_ns_c", "bx_c", "bx8_c", "bxp_c",
                     "bf1_c", "bfg_c",
                     "bv_bc", "exptab", "bd1_c", "ln3_s_c", "ln3_ns_c",
                     "ln3_b_c", "wd2", "bd2_c", "wout", "bout_t", "otherT"):
            ap = t[name]
            tl = pp.tile(list(ap.shape), ap.dtype, name=f"c_{name}")
            nc.gpsimd.dma_start(out=tl[:], in_=ap[:])
            consts[name] = tl
        c.consts = consts

        with tc.tile_pool(name="wlayer", bufs=1) as wp:
            c.wp = wp
            # layer-0 big weights start streaming before the input proj
            w5 = _load_layer_weights(nc, c, 0)

            # ---- input projection: xT / xq8 ----
            c.xq = [None] * NG
            with tc.tile_pool(name="inp", bufs=1) as inp:
                cgmT_s = inp.tile([DFEAT, NTOK], BF16, name="cgm_s")
                nc.sync.dma_start(out=cgmT_s[:], in_=t["cgmT"][:])
                w_in_s = inp.tile([DFEAT, D], BF16, name="w_in_s")
                nc.sync.dma_start(out=w_in_s[:], in_=t["w_in"][:])
                for g in range(NG):
                    tok = slice(g * GT, (g + 1) * GT)
                    xq_g = xqp.tile([128, DC, GT], FP8, tag="xq", name="xq_g")
                    for dd in range(DC):
                        ps = mmp.tile([128, GT], F32, tag="mm", name="ps_in")
                        nc.tensor.matmul(
                            ps[:], w_in_s[:, dd * 128:(dd + 1) * 128],
                            cgmT_s[:, tok], start=True, stop=True)
                        nc.vector.tensor_scalar_add(
                            out=c.xT[:, dd, tok], in0=ps[:],
                            scalar1=consts["b_in_c"][:, dd:dd + 1])
                        nc.scalar.activation(
                            out=xq_g[:, dd, :], in_=ps[:], func=AF.Identity,
                            bias=consts["b_in8_c"][:, dd:dd + 1], scale=XS)
                    c.xq[g] = xq_g

            # ---- transformer layers ----
            for i in range(NL):
                w5_next = _layer(nc, tc, c, i, w5)
                w5 = w5_next

        # ---- head ----
        _head(nc, tc, c, y_out)


def _load_layer_weights(nc, c, i):
    """Preload the V weight whole-layer; Q/K/O/G stream in per-chunk."""
    w5 = {}
    for nm in ("wv8",):
        tl = c.wp.tile([128, DC, D], FP8, tag=nm, name=f"{nm}_s")
        nc.sync.dma_start(out=tl[:], in_=c.t[nm][i])
        w5[nm] = tl
    return w5


_CHUNK_BUFS = {"wq8": 6, "wk8": 6, "wg8": 4, "wo8": 4}


def _w_chunk(nc, c, nm, i, dd, queue="sync"):
    ch = c.wp.tile([128, DC, 128], FP8, tag=f"{nm}_ch", bufs=_CHUNK_BUFS[nm],
                   name=f"{nm}_ch")
    eng = nc.sync if queue == "sync" else nc.gpsimd
    eng.dma_start(out=ch[:], in_=c.t[nm][i, :, :, dd * 128:(dd + 1) * 128])
    return ch


def _layer(nc, tc, c, i, w5):
    with (
        tc.tile_pool(name="grp", bufs=2) as gp,
        tc.tile_pool(name="attw", bufs=2) as at,
    ):
        c.gp, c.at = gp, at
        qkv = [None, None]   # rotating (q8, k8, v8) per group parity
        qkv[0] = _emit_qkv(nc, c, i, w5, 0, fillers=None)

        sig_t = None
        for g in range(NG):
            fillers = []
            if g + 1 < NG:
                qkv[(g + 1) % 2] = _emit_qkv(nc, c, i, w5, g + 1,
                                             fillers=fillers)
            else:
                # last group: the gate projection (which only needs xq)
                # becomes the filler material
                sig_t = _emit_gate(nc, c, i, w5, g, fillers=fillers)
            ao8 = _attention(nc, c, i, g, qkv[g % 2], fillers)
            _og_ln1(nc, c, i, w5, g, ao8, qkv[g % 2],
                    sig_pre=sig_t if g == NG - 1 else None)

    # layer i+1 weights stream during this layer's FFN
    w5_next = _load_layer_weights(nc, c, i + 1) if i + 1 < NL else None

    _ffn(nc, tc, c, i)
    return w5_next


def _emit_qkv(nc, c, i, w5, g, fillers):
    """Q/K/V projections for group g. If fillers is a list, append one
    closure per PSUM-group instead of emitting directly."""
    consts = c.consts
    tok = slice(g * GT, (g + 1) * GT)
    q8 = c.gp.tile([128, H, GT], FP8, tag="q8", name="q8")
    k8 = c.gp.tile([128, H, GT], FP8, tag="k8", name="k8")
    v8 = c.gp.tile([128, GB, D], FP8, tag="v8", name="v8")
    xq = c.xq[g]

    def q_chunk(dd):
        def f():
            wq_ch = _w_chunk(nc, c, "wq8", i, dd)
            ps = c.mmp.tile([128, GT], F32, tag="mm", name="psq")
            for kc in range(DC2):
                nc.tensor.matmul(
                    ps[:], wq_ch[:, 2 * kc:2 * kc + 2, :],
                    xq[:, 2 * kc:2 * kc + 2, :],
                    start=(kc == 0), stop=(kc == DC2 - 1), perf_mode=DR)
            nc.vector.tensor_scalar(
                out=q8[:, dd, :], in0=ps[:],
                scalar1=QS * QSCALE / (WS * XS),
                scalar2=consts["bq_c"][:, i, dd:dd + 1],
                op0=OP.mult, op1=OP.add)
        return f

    def k_chunk(dd):
        def f():
            wk_ch = _w_chunk(nc, c, "wk8", i, dd)
            ps = c.mmp.tile([128, GT], F32, tag="mm", name="psk")
            for kc in range(DC2):
                nc.tensor.matmul(
                    ps[:], wk_ch[:, 2 * kc:2 * kc + 2, :],
                    xq[:, 2 * kc:2 * kc + 2, :],
                    start=(kc == 0), stop=(kc == DC2 - 1), perf_mode=DR)
            nc.vector.tensor_scalar(
                out=k8[:, dd, :], in0=ps[:],
                scalar1=QS / (WS * XS),
                scalar2=consts["bk_c"][:, i, dd:dd + 1],
                op0=OP.mult, op1=OP.add)
        return f

    def v_chunk(cc, jj):
        def f():
            ps = c.mmp.tile([128, 512], F32, tag="mm", name="psv")
            for kc in range(DC2):
                nc.tensor.matmul(
                    ps[:], xq[:, 2 * kc:2 * kc + 2, jj * L:(jj + 1) * L],
                    w5["wv8"][:, 2 * kc:2 * kc + 2,
                              cc * 512:(cc + 1) * 512],
                    start=(kc == 0), stop=(kc == DC2 - 1), perf_mode=DR)
            nc.vector.scalar_tensor_tensor(
                out=v8[:, jj, cc * 512:(cc + 1) * 512], in0=ps[:],
                scalar=VS / (WS * XS),
                in1=consts["bv_bc"][:, i, cc * 512:(cc + 1) * 512],
                op0=OP.mult, op1=OP.add)
        return f

    closures = ([q_chunk(dd) for dd in range(DC)]
                + [k_chunk(dd) for dd in range(DC)]
                + [v_chunk(cc, jj) for cc in range(2) for jj in range(GB)])
    if fillers is None:
        for f in closures:
            f()
    else:
        fillers.extend(closures)
    return q8, k8, v8


def _emit_gate(nc, c, i, w5, g, fillers):
    """Gate projection sigmoid(x@Wg+bg) for group g (filler closures)."""
    consts = c.consts
    xq = c.xq[g]
    sig_t = c.rp.tile([128, DC, GT], BF16, tag="sig", name="sig_g")

    def g_chunk(dd):
        def f():
            wg_ch = _w_chunk(nc, c, "wg8", i, dd, queue="gpsimd")
            ps = c.mmp.tile([128, GT], F32, tag="mm", name="psg")
            for kc in range(DC2):
                nc.tensor.matmul(
                    ps[:], wg_ch[:, 2 * kc:2 * kc + 2, :],
                    xq[:, 2 * kc:2 * kc + 2, :],
                    start=(kc == 0), stop=(kc == DC2 - 1), perf_mode=DR)
            nc.scalar.activation(
                out=sig_t[:, dd, :], in_=ps[:], func=AF.Sigmoid,
                bias=consts["bg_c"][:, i, dd:dd + 1], scale=1.0 / (WS * XS))
        return f

    fillers.extend(g_chunk(dd) for dd in range(DC))
    return sig_t


def _attention(nc, c, i, g, qkv, fillers):
    """Attention for group g, one wave per batch (8 heads in 2 half-waves).
    Emits filler closures between dependency steps to keep the PE dense."""
    q8, k8, v8 = qkv
    fi = iter(fillers)

    def pump(n):
        for _ in range(n):
            f = next(fi, None)
            if f is None:
                return
            f()

    ao8 = c.gp.tile([128, H, GB, L], FP8, tag="ao8", bufs=1, name="ao8")
    for jj in range(GB):
        b_local = g * GB + jj
        jtok = slice(jj * L, (jj + 1) * L)
        etab = c.consts["exptab"][:, i, 15 - b_local:143 - b_local]
        pa = [None, None]
        awe2 = [None, None]
        for half in range(2):
            h0 = half * 4
            pa[half] = c.app.tile([128, 512], F32, tag="pa", name="pa")
            for hh in range(4):
                nc.tensor.matmul(
                    pa[half][:, hh * L:(hh + 1) * L],
                    k8[:, h0 + hh, jtok], q8[:, h0 + hh, jtok],
                    start=True, stop=True)
        pump(2)
        rb = [None, None]
        for half in range(2):
            awe = c.at.tile([128, 512], BF16, tag="awe", name="awe")
            nc.scalar.activation(out=awe[:], in_=pa[half][:], func=AF.Exp,
                                 scale=1.0 / (QS * QS))
            # in-place multiply by the per-batch exp(bias) table
            nc.vector.tensor_tensor(
                out=awe[:].rearrange("p (h q) -> p h q", q=L),
                in0=awe[:].rearrange("p (h q) -> p h q", q=L),
                in1=etab.unsqueeze(1).to_broadcast([128, 4, L]), op=OP.mult)
            awe2[half] = awe
        pump(2)
        for half in range(2):
            # softmax sums land in partition 0 of the (already-read) logit
            # psum tile — saves a PSUM bank
            sm = pa[half][0:1, :]
            nc.tensor.matmul(sm, c.ones_col_bf[:, 0:1], awe2[half][:],
                             start=True, stop=True)
            smr = c.at.tile([1, 512], F32, tag="smr", bufs=1, name="smr")
            nc.scalar.copy(out=smr[:], in_=sm)
            sb = c.lnp.tile([128, 512], F32, tag="rs", name="sb")
            nc.gpsimd.partition_broadcast(sb[:], smr[:])
            rb[half] = c.lnp.tile([128, 512], F32, tag="mrs", name="rb")
            nc.vector.reciprocal_approx_fast(out=rb[half][:], in_=sb[:])
        pump(2)
        for half in range(2):
            h0 = half * 4
            pao = c.app.tile([128, 512], F32, tag="pa", name="pao")
            for hh in range(4):
                nc.tensor.matmul(
                    pao[:, hh * L:(hh + 1) * L],
                    v8[:, jj, (h0 + hh) * DK:(h0 + hh + 1) * DK],
                    awe2[half][:, hh * L:(hh + 1) * L], start=True, stop=True)
            nc.vector.tensor_tensor(
                out=ao8[:, h0:h0 + 4, jj, :],
                in0=pao[:].rearrange("p (h q) -> p h q", q=L),
                in1=rb[half][:].rearrange("p (h q) -> p h q", q=L),
                op=OP.mult)
    # drain any remaining fillers
    pump(1 << 30)
    return ao8


def _og_ln1(nc, c, i, w5, g, ao8, qkv, sig_pre):
    """Gate + O-projection + residual + LN1 for group g."""
    consts = c.consts
    tok = slice(g * GT, (g + 1) * GT)
    xq = c.xq[g]

    # gate sigmoid (unless pre-emitted as attention fillers for the last
    # group); emitting it here keeps the scalar queue clear of sigmoids
    # during the attention waves and the psum rotation unclogged
    if sig_pre is None:
        sig_t = c.rp.tile([128, DC, GT], BF16, tag="sig", name="sig_g")
        for dd in range(DC):
            wg_ch = _w_chunk(nc, c, "wg8", i, dd, queue="gpsimd")
            ps = c.mmp.tile([128, GT], F32, tag="mm", name="psg")
            for kc in range(DC2):
                nc.tensor.matmul(
                    ps[:], wg_ch[:, 2 * kc:2 * kc + 2, :],
                    xq[:, 2 * kc:2 * kc + 2, :],
                    start=(kc == 0), stop=(kc == DC2 - 1), perf_mode=DR)
            nc.scalar.activation(
                out=sig_t[:, dd, :], in_=ps[:], func=AF.Sigmoid,
                bias=consts["bg_c"][:, i, dd:dd + 1], scale=1.0 / (WS * XS))
    else:
        sig_t = sig_pre

    res_t = c.rp.tile([128, DC, GT], BF16, tag="res", name="res_t")
    ps_s = c.stp.tile([128, GT], F32, tag="ss", name="ps_s")
    ps_q = c.stp.tile([128, GT], F32, tag="sq", name="ps_q")
    for dd in range(DC):
        wo_ch = _w_chunk(nc, c, "wo8", i, dd, queue="gpsimd")
        pso = c.mmp.tile([128, GT], F32, tag="mm", name="pso")
        for kc in range(DC2):
            nc.tensor.matmul(
                pso[:], wo_ch[:, 2 * kc:2 * kc + 2, :],
                ao8[:, 2 * kc:2 * kc + 2, :, :],
                start=(kc == 0), stop=(kc == DC2 - 1), perf_mode=DR)
        t1 = c.sqp.tile([128, GT], F32, tag="t1", bufs=1, name="t1")
        nc.vector.tensor_scalar(
            out=t1[:], in0=pso[:], scalar1=1.0 / (WS * VS),
            scalar2=consts["bo_c"][:, i, dd:dd + 1], op0=OP.mult, op1=OP.add)
        nc.vector.tensor_mul(out=t1[:], in0=t1[:], in1=sig_t[:, dd, :])
        # res = (xT_nob + bx) + gated-attn; bx folds the previous layer's
        # deferred LN2 bias (zero for layer 0, whose xT carries b_in)
        nc.vector.scalar_tensor_tensor(
            out=res_t[:, dd, :], in0=c.xT[:, dd, tok],
            scalar=consts["bx_c"][:, i, dd:dd + 1], in1=t1[:],
            op0=OP.add, op1=OP.add)
        sq = c.sqp.tile([128, GT], BF16, tag="sq", name="sq")
        nc.scalar.activation(out=sq[:], in_=res_t[:, dd, :], func=AF.Square)
        nc.tensor.matmul(ps_s[:], c.ones128[:], res_t[:, dd, :],
                         start=(dd == 0), stop=(dd == DC - 1))
        nc.tensor.matmul(ps_q[:], c.ones128[:], sq[:],
                         start=(dd == 0), stop=(dd == DC - 1))

    rs_f, mrs_f = _ln_rows_full(nc, c, ps_s, ps_q, GT, 1.0 / D)
    for dd in range(DC):
        u = c.lnp.tile([128, GT], F32, tag="u", bufs=1, name="u")
        nc.vector.scalar_tensor_tensor(
            out=u[:], in0=res_t[:, dd, :],
            scalar=consts["ln1_s_c"][:, i, dd:dd + 1], in1=rs_f[:],
            op0=OP.mult, op1=OP.mult)
        # h is stored WITHOUT ln1_b; the bias is folded into the two
        # consumers (h8 shadow below, f2-residual add in _ffn)
        nc.vector.scalar_tensor_tensor(
            out=c.h[:, dd, tok], in0=mrs_f[:],
            scalar=consts["ln1_ns_c"][:, i, dd:dd + 1], in1=u[:],
            op0=OP.mult, op1=OP.add)
        nc.scalar.activation(out=c.h8[:, dd, tok], in_=c.h[:, dd, tok],
                             func=AF.Identity,
                             bias=consts["ln1_b8_c"][:, i, dd:dd + 1],
                             scale=XS)


def _ln_rows_full(nc, c, ps_s, ps_q, n, inv_d):
    """Full-width LN stats: rs = 1/sqrt(var+eps), mrs = mean*rs as
    [128, n] tiles (all rows identical)."""
    m_f = c.lnp.tile([128, 512], F32, tag="m", bufs=1, name="m_f")[:, :n]
    nc.vector.tensor_scalar_mul(out=m_f, in0=ps_s[:], scalar1=inv_d)
    m2 = c.lnp.tile([128, 512], F32, tag="tmp", bufs=2, name="m2")[:, :n]
    nc.vector.tensor_mul(out=m2, in0=m_f, in1=m_f)
    var = c.lnp.tile([128, 512], F32, tag="tmp", bufs=2, name="var")[:, :n]
    nc.vector.scalar_tensor_tensor(out=var, in0=ps_q[:], scalar=inv_d,
                                   in1=m2, op0=OP.mult, op1=OP.subtract)
    std = c.lnp.tile([128, 512], F32, tag="tmp", bufs=2, name="std")[:, :n]
    nc.scalar.activation(out=std, in_=var, func=AF.Sqrt, bias=c.eps_col[:],
                         scale=1.0)
    rs_f = c.lnp.tile([128, 512], F32, tag="rs", name="rs_f")[:, :n]
    nc.vector.reciprocal_approx_fast(out=rs_f, in_=std)
    mrs_f = c.lnp.tile([128, 512], F32, tag="mrs", name="mrs_f")[:, :n]
    nc.vector.tensor_mul(out=mrs_f, in0=m_f, in1=rs_f)
    return rs_f, mrs_f


def _ffn(nc, tc, c, i):
    consts = c.consts
    with (
        tc.tile_pool(name="fbuf", bufs=1) as fp,
        tc.tile_pool(name="fwch", bufs=4) as wc,
        tc.tile_pool(name="fw2ch", bufs=2) as wc2,
    ):
        for sub in range(NG):
            tok = slice(sub * GT, (sub + 1) * GT)
            f8 = fp.tile([128, FC, GT], FP8, tag="f8", name="f8")
            # --- f8 = (h@Wf1 + bf1) * sigmoid(h@Wfg + bfg), fp8-scaled ---
            for fc in range(FC):
                if fc % 8 == 4 and c.pool_pending:
                    c.pool_pending.pop(0)()
                wf1_ch = wc.tile([128, DC, 128], FP8, tag="wf1", bufs=4, name="wf1c")
                nc.sync.dma_start(
                    out=wf1_ch[:],
                    in_=c.t["wf18"][i, :, :, fc * 128:(fc + 1) * 128])
                wfg_ch = wc.tile([128, DC, 128], FP8, tag="wfg", bufs=4, name="wfgc")
                nc.gpsimd.dma_start(
                    out=wfg_ch[:],
                    in_=c.t["wfg8"][i, :, :, fc * 128:(fc + 1) * 128])
                ps1 = c.mmp.tile([128, GT], F32, tag="mm", name="ps1")
                psg = c.mmp.tile([128, GT], F32, tag="mm", name="psfg")
                for kc in range(DC2):
                    nc.tensor.matmul(
                        ps1[:], wf1_ch[:, 2 * kc:2 * kc + 2, :],
                        c.h8[:, 2 * kc:2 * kc + 2, tok],
                        start=(kc == 0), stop=(kc == DC2 - 1), perf_mode=DR)
                for kc in range(DC2):
                    nc.tensor.matmul(
                        psg[:], wfg_ch[:, 2 * kc:2 * kc + 2, :],
                        c.h8[:, 2 * kc:2 * kc + 2, tok],
                        start=(kc == 0), stop=(kc == DC2 - 1), perf_mode=DR)
                sig = c.sqp.tile([128, GT], BF16, tag="fsig", name="fsig")
                nc.scalar.activation(
                    out=sig[:], in_=psg[:], func=AF.Sigmoid,
                    bias=consts["bfg_c"][:, i, fc:fc + 1],
                    scale=1.0 / (WS * XS))
                nc.vector.scalar_tensor_tensor(
                    out=f8[:, fc, :], in0=ps1[:],
                    scalar=consts["bf1_c"][:, i, fc:fc + 1], in1=sig[:],
                    op0=OP.add, op1=OP.mult)

            # --- f @ Wf2 + bf2 + h, LN2, stochastic-depth blend into xT ---
            res_t = c.rp.tile([128, DC, GT], BF16, tag="res", name="res2")
            ps_s = c.stp.tile([128, GT], F32, tag="ss", name="ps_s2")
            ps_q = c.stp.tile([128, GT], F32, tag="sq", name="ps_q2")
            for dd in range(DC):
                wf2_ch = wc2.tile([128, FC, 128], FP8, tag="wf2", name="wf2c")
                nc.gpsimd.dma_start(
                    out=wf2_ch[:],
                    in_=c.t["wf28"][i, :, :, dd * 128:(dd + 1) * 128])
                ps2 = c.mmp.tile([128, GT], F32, tag="mm", name="ps2")
                for fc in range(FC2):
                    nc.tensor.matmul(
                        ps2[:], wf2_ch[:, 2 * fc:2 * fc + 2, :],
                        f8[:, 2 * fc:2 * fc + 2, :],
                        start=(fc == 0), stop=(fc == FC2 - 1), perf_mode=DR)
                t1 = c.sqp.tile([128, GT], F32, tag="t1", bufs=1, name="ft1")
                nc.vector.tensor_scalar(
                    out=t1[:], in0=ps2[:], scalar1=1.0 / (WS * WS_F1 * XS),
                    scalar2=consts["bf2_c"][:, i, dd:dd + 1],
                    op0=OP.mult, op1=OP.add)
                # h is stored without ln1_b; add it back here
                nc.vector.scalar_tensor_tensor(
                    out=res_t[:, dd, :], in0=c.h[:, dd, tok],
                    scalar=consts["ln1_b_c"][:, i, dd:dd + 1], in1=t1[:],
                    op0=OP.add, op1=OP.add)
                sq = c.sqp.tile([128, GT], BF16, tag="sq", name="fsq")
                nc.scalar.activation(out=sq[:], in_=res_t[:, dd, :],
                                     func=AF.Square)
                nc.tensor.matmul(ps_s[:], c.ones128[:], res_t[:, dd, :],
                                 start=(dd == 0), stop=(dd == DC - 1))
                nc.tensor.matmul(ps_q[:], c.ones128[:], sq[:],
                                 start=(dd == 0), stop=(dd == DC - 1))

            rs_f, mrs_f = _ln_rows_full(nc, c, ps_s, ps_q, GT, 1.0 / D)
            sv = SURVIVE[i]
            for dd in range(DC):
                u = c.lnp.tile([128, GT], F32, tag="u", bufs=1, name="fu")
                nc.vector.scalar_tensor_tensor(
                    out=u[:], in0=res_t[:, dd, :],
                    scalar=consts["ln2_s_c"][:, i, dd:dd + 1], in1=rs_f[:],
                    op0=OP.mult, op1=OP.mult)
                if sv == 1.0:
                    nc.vector.scalar_tensor_tensor(
                        out=c.xT[:, dd, tok], in0=mrs_f[:],
                        scalar=consts["ln2_ns_c"][:, i, dd:dd + 1], in1=u[:],
                        op0=OP.mult, op1=OP.add)
                else:
                    v1 = c.lnp.tile([128, GT], F32, tag="v1", bufs=1,
                                    name="fv1")
                    nc.vector.scalar_tensor_tensor(
                        out=v1[:], in0=mrs_f[:],
                        scalar=consts["ln2_ns_c"][:, i, dd:dd + 1], in1=u[:],
                        op0=OP.mult, op1=OP.add)
                    nc.vector.scalar_tensor_tensor(
                        out=c.xT[:, dd, tok], in0=c.xT[:, dd, tok],
                        scalar=1.0 - sv, in1=v1[:], op0=OP.mult, op1=OP.add)
                # xT is stored WITHOUT the (survive-scaled) ln2 bias; it is
                # folded into the consumers (xq shadow, next layer's
                # O-residual via bx_c, pooling via bxp_c)
                if i + 1 < NL:
                    # fp8 shadow for the next layer's projections
                    if dd == 0:
                        c.xq[sub] = c.xqp.tile([128, DC, GT], FP8, tag="xq",
                                               name="xq_n")
                    nc.scalar.activation(out=c.xq[sub][:, dd, :],
                                         in_=c.xT[:, dd, tok],
                                         func=AF.Identity,
                                         bias=consts["bx8_c"][:, i + 1,
                                                             dd:dd + 1],
                                         scale=XS)
            if i + 1 == NL:
                # head pooling: enqueue one small reduce per batch; they are
                # drained inside the next sub's fc loop so the 1.2us vector
                # ops never block the PE's psum drains in a burst
                if sub == 0:
                    c.pool_sum = c.lnp.tile([128, DC, BPC], F32, tag="pls",
                                            bufs=1, name="pool_sum")
                    c.pool_max = c.lnp.tile([128, DC, BPC], F32, tag="plm",
                                            bufs=1, name="pool_max")

                def mk_pool(b_abs):
                    def f():
                        xv = c.xT[:, :, b_abs * L:(b_abs + 1) * L]
                        nc.vector.tensor_reduce(
                            out=c.pool_sum[:, :, b_abs], in_=xv,
                            axis=mybir.AxisListType.X, op=OP.add)
                        nc.vector.tensor_reduce(
                            out=c.pool_max[:, :, b_abs], in_=xv,
                            axis=mybir.AxisListType.X, op=OP.max)
                    return f
                for jj in range(GB):
                    c.pool_pending.append(mk_pool(sub * GB + jj))


def _head(nc, tc, c, y_out):
    consts = c.consts
    with tc.tile_pool(name="head", bufs=1) as hp:
        for f in c.pool_pending:
            f()
        c.pool_pending = []
        wd1_s = hp.tile([128, 17, 128], BF16, name="wd1_s")
        nc.sync.dma_start(out=wd1_s[:], in_=c.t["wd1"][:])
        poolT = hp.tile([128, 17, BPC], BF16, name="poolT")
        for dd in range(DC):
            # mean/max pooling of xT_nob + deferred final LN2 bias (bxp)
            nc.vector.tensor_scalar(
                out=poolT[:, dd, :], in0=c.pool_sum[:, dd, :],
                scalar1=1.0 / L, scalar2=consts["bxp_c"][:, dd:dd + 1],
                op0=OP.mult, op1=OP.add)
            nc.vector.tensor_scalar_add(
                out=poolT[:, DC + dd, :], in0=c.pool_max[:, dd, :],
                scalar1=consts["bxp_c"][:, dd:dd + 1])
        nc.gpsimd.tensor_copy(out=poolT[:, 16, :], in_=consts["otherT"][:])

        # y1 = relu(pooled @ Wd1 + bd1)   [128 dout, 16]
        ps1 = c.mmp.tile([128, GT], F32, tag="mm", name="hps")[:, 0:BPC]
        for cc in range(17):
            nc.tensor.matmul(ps1, wd1_s[:, cc, :], poolT[:, cc, :],
                             start=(cc == 0), stop=(cc == 16))
        y1 = hp.tile([128, BPC], mybir.dt.float32r, name="y1")
        with nc.allow_low_precision(reason="fp32r head LN within tolerance"):
            nc.scalar.activation(out=y1[:], in_=ps1, func=AF.Relu,
                                 bias=consts["bd1_c"][:], scale=1.0)

        # LN3 over the 128 features (partition dim), full-width stats
        sq3 = hp.tile([128, BPC], mybir.dt.float32r, name="sq3")
        with nc.allow_low_precision(reason="fp32r head LN within tolerance"):
            nc.vector.tensor_mul(out=sq3[:], in0=y1[:].bitcast(F32),
                                 in1=y1[:].bitcast(F32))
        ps_s = c.stp.tile([128, GT], F32, tag="ss", name="hs")[:, 0:BPC]
        ps_q = c.stp.tile([128, GT], F32, tag="sq", name="hq")[:, 0:BPC]
        nc.tensor.matmul(ps_s, c.ones128r[:], y1[:], start=True, stop=True)
        nc.tensor.matmul(ps_q, c.ones128r[:], sq3[:], start=True, stop=True)
        rs_f, mrs_f = _ln_rows_full(nc, c, ps_s, ps_q, BPC, 1.0 / 128)
        u3 = hp.tile([128, BPC], F32, name="u3")
        nc.vector.scalar_tensor_tensor(
            out=u3[:], in0=y1[:].bitcast(F32), scalar=consts["ln3_s_c"][:],
            in1=rs_f[:], op0=OP.mult, op1=OP.mult)
        yln0 = hp.tile([128, BPC], F32, name="yln0")
        nc.vector.scalar_tensor_tensor(
            out=yln0[:], in0=mrs_f[:], scalar=consts["ln3_ns_c"][:],
            in1=u3[:], op0=OP.mult, op1=OP.add)
        yln = hp.tile([128, BPC], BF16, name="yln")
        nc.vector.tensor_scalar_add(out=yln[:], in0=yln0[:],
                                    scalar1=consts["ln3_b_c"][:])

        # y2 = relu(yln @ Wd2 + bd2); y = y2 @ Wout + bout
        ps2 = c.mmp.tile([128, GT], F32, tag="mm", name="hps2")[:, 0:BPC]
        nc.tensor.matmul(ps2, consts["wd2"][:], yln[:], start=True, stop=True)
        y2 = hp.tile([128, BPC], BF16, name="y2")
        nc.scalar.activation(out=y2[:], in_=ps2, func=AF.Relu,
                             bias=consts["bd2_c"][:], scale=1.0)
        psy = c.mmp.tile([128, GT], F32, tag="mm", name="hpsy")[0:1, 0:BPC]
        nc.tensor.matmul(psy, consts["wout"][:], y2[:], start=True, stop=True)
        yfin = hp.tile([1, BPC], F32, name="yfin")
        nc.vector.tensor_tensor(
            out=yfin[:], in0=psy,
            in1=consts["bout_t"][:].to_broadcast([1, BPC]), op=OP.add)
        nc.sync.dma_start(out=y_out[:], in_=yfin[:])


# ---------------------------------------------------------------------------
# host side
# ---------------------------------------------------------------------------

def _bf(x):
    return np.ascontiguousarray(np.asarray(x, np.float32)).astype(
        ml_dtypes.bfloat16)


def _f32(x):
    return np.ascontiguousarray(np.asarray(x, np.float32))


def _fp8(x, scale):
    a = np.asarray(x, np.float32) * np.float32(scale)
    a = np.clip(a, -240.0, 240.0)
    return np.ascontiguousarray(a).astype(ml_dtypes.float8_e4m3)


def _prep_shared(I):
    sv = np.array(SURVIVE, np.float32)
    s = {}
    s["w_in"] = _bf(I["W_in"])
    b_in = np.asarray(I["b_in"], np.float32).reshape(DC, 128).T
    s["b_in_c"] = _f32(b_in)
    s["b_in8_c"] = _f32(b_in * XS)
    for nm, W, ws in (("wq8", I["Wq"], WS), ("wk8", I["Wk"], WS),
                      ("wv8", I["Wv"], WS), ("wg8", I["Wg"], WS),
                      ("wo8", I["Wo"], WS)):
        s[nm] = _fp8(np.asarray(W, np.float32).reshape(NL, DC, 128, D)
                     .transpose(0, 2, 1, 3), ws)
    s["wf18"] = _fp8(np.asarray(I["Wf1"], np.float32)
                     .reshape(NL, DC, 128, FF).transpose(0, 2, 1, 3), WS_F1)
    s["wfg8"] = _fp8(np.asarray(I["Wfg"], np.float32)
                     .reshape(NL, DC, 128, FF).transpose(0, 2, 1, 3), WS)
    s["wf28"] = _fp8(np.asarray(I["Wf2"], np.float32)
                     .reshape(NL, FC, 128, D).transpose(0, 2, 1, 3), WS)

    def col(b, nch):
        return _f32(np.asarray(b, np.float32).reshape(NL, nch, 128)
                    .transpose(2, 0, 1))

    s["bq_c"] = col(np.asarray(I["bq"], np.float32) * (QS * QSCALE), DC)
    s["bk_c"] = col(np.asarray(I["bk"], np.float32) * QS, DC)
    s["bg_c"] = col(I["bg"], DC)
    s["bo_c"] = col(I["bo"], DC)
    s["bf2_c"] = col(I["bf2"], DC)
    s["bf1_c"] = col(np.asarray(I["bf1"], np.float32) * (WS_F1 * XS), FC)
    s["bfg_c"] = col(I["bfg"], FC)
    ln1_s = np.asarray(I["ln1_s"], np.float32)
    ln2_s = np.asarray(I["ln2_s"], np.float32) * sv[:, None]
    s["ln1_s_c"] = col(ln1_s, DC)
    s["ln1_ns_c"] = col(-ln1_s, DC)
    s["ln1_b_c"] = col(I["ln1_b"], DC)
    s["ln1_b8_c"] = col(np.asarray(I["ln1_b"], np.float32) * XS, DC)
    s["ln2_s_c"] = col(ln2_s, DC)
    s["ln2_ns_c"] = col(-ln2_s, DC)
    # deferred (survive-scaled) LN2 bias for the residual stream:
    # bx[i] = bias carried by xT entering layer i (0 for layer 0 — its xT
    # includes b_in directly); bxp = bias at pooling time.
    ln2_b = np.asarray(I["ln2_b"], np.float32)
    bx = np.zeros((NL, D), np.float32)
    running = np.zeros(D, np.float32)
    for li in range(NL):
        bx[li] = running
        running = (1.0 - sv[li]) * running + sv[li] * ln2_b[li]
    s["bx_c"] = col(bx, DC)
    s["bx8_c"] = col(bx * XS, DC)
    s["bxp_c"] = _f32(running.reshape(DC, 128).T)
    s["bv_bc"] = _fp8(np.broadcast_to(
        np.asarray(I["bv"], np.float32)[None] * VS, (128, NL, D)), 1.0)
    s["wd1"] = _bf(np.concatenate(
        [np.asarray(I["Wd1"], np.float32),
         np.zeros((17 * 128 - I["Wd1"].shape[0], 128), np.float32)],
        axis=0).reshape(17, 128, 128).transpose(1, 0, 2))
    s["bd1_c"] = _f32(I["bd1"].reshape(128, 1))
    ln3_s = np.asarray(I["ln3_s"], np.float32)
    s["ln3_s_c"] = _f32(ln3_s.reshape(128, 1))
    s["ln3_ns_c"] = _f32(-ln3_s.reshape(128, 1))
    s["ln3_b_c"] = _f32(I["ln3_b"].reshape(128, 1))
    s["wd2"] = _bf(I["Wd2"])
    s["bd2_c"] = _f32(I["bd2"].reshape(128, 1))
    s["wout"] = _bf(I["Wout"])
    s["bout_t"] = _f32(I["bout"].reshape(1, 1))
    return s


def _prep_core(I, shared, cidx):
    m = dict(shared)
    cgm = np.asarray(I["cgm"], np.float32)
    m["cgmT"] = _bf(cgm[cidx * BPC:(cidx + 1) * BPC].reshape(NTOK, DFEAT).T)
    oth = np.asarray(I["other"], np.float32)[cidx * BPC:(cidx + 1) * BPC]
    m["otherT"] = _bf(np.concatenate(
        [oth.T, np.zeros((128 - OTHER, BPC), np.float32)], axis=0))
    # exp of transposed+flipped rel-pos bias table, per-core column window
    rel = np.asarray(I["rel_emb"], np.float32)          # [NL, 255, 128]
    flippedT = rel[:, ::-1, :].transpose(0, 2, 1)       # [NL, 128, 255]
    lo = 112 - 16 * cidx
    tab = flippedT[:, :, lo:lo + 143]                   # [NL, 128, 143]
    m["exptab"] = _bf(np.exp(tab).transpose(1, 0, 2))   # [128, NL, 143]
    return m


def kernel(**inputs) -> np.ndarray:
    if "nc" not in _cached:
        _cached["nc"] = _build_nc()
    nc = _cached["nc"]
    shared = _prep_shared(inputs)
    in_maps = [_prep_core(inputs, shared, cidx) for cidx in range(NCORES)]
    res = run_bass_kernel_spmd(nc, in_maps, core_ids=list(range(NCORES)))
    y = np.concatenate([res.results[cidx]["y"].reshape(BPC)
                        for cidx in range(NCORES)])
    return y.reshape(B, 1).astype(np.float32)


# revision 55
# speedup vs baseline: 1.1767x; 1.0532x over previous
"""Trainium2 Bass kernel for nn_AttentionModel_63737314672806.

Sharding: data-parallel over batch (B=128) across 8 NeuronCores; each core
processes 16 batch elements (2048 tokens) through the full model. Weights are
replicated (broadcast) to every core. No collectives.

Layout: activations are feature-major ("transposed"):
  xT[p, c, t] = x[token t, feature c*128+p]
so dense layers are psum = matmul(lhsT=W[kc, dout], rhs=xT[kc, tok]) with the
output feature-major again. All heavy matmuls (QKVGO projections and the FFN)
run in fp8 e4m3 with perf_mode=DoubleRow (2 fp8 MACs/cell/cycle): weights are
host-quantized with static power-of-2 scales, activations get fp8 shadow
copies on device, and descales fold into the existing post-PSUM vector ops.
Attention itself stays in fp8/bf16 normal-mode matmuls with the
host-precomputed exp(bias) table trick; softmax runs 4 heads at a time in
[128,512] tiles. LayerNorm statistics are computed full-width ([128,512]
ones-matmuls) so all row math runs on 128 vector lanes and no PE rank-1
broadcasts are needed. The residual streams (xT, h) stay bf16 in SBUF for
accuracy; there is no DRAM round-trip.

PE density: attention's latency chains are interleaved at emission time with
the next group's Q/K/V projection matmuls (and with the gate projection for
the last group) so the in-order PE queue always has dense work and the HAM
clock gate stays warm. Head pooling is done incrementally per FFN sub-block.
"""

import math

import numpy as np
import ml_dtypes

import concourse.bass as bass
import concourse.bacc as bacc
import concourse.mybir as mybir
import concourse.tile as tile
from concourse.bass_utils import run_bass_kernel_spmd

BF16 = mybir.dt.bfloat16
F32 = mybir.dt.float32
FP8 = mybir.dt.float8e4
AF = mybir.ActivationFunctionType
OP = mybir.AluOpType
DR = mybir.MatmulPerfMode.DoubleRow

NCORES = 8
B = 128
L = 128
DFEAT = 32
H = 8
DK = 128
D = 1024
FF = 4096
NL = 2
MAXPOS = 128
OTHER = 64
EPS = 1e-6

BPC = B // NCORES       # 16 batches per core
NTOK = BPC * L          # 2048 tokens per core
NG = 4                  # batch groups per core
GB = BPC // NG          # batches per group = 4
GT = GB * L             # tokens per group = 512
DC = D // 128           # 8 feature chunks
DC2 = DC // 2           # 4 DoubleRow chunk-pairs
FC = FF // 128          # 32 ff chunks
FC2 = FC // 2           # 16 DoubleRow pairs
QSCALE = 1.0 / math.sqrt(float(DK))
SURVIVE = [1.0, 0.5]

# static fp8 scales (powers of two; see docstring)
WS = 2.0 ** 7           # weight scale: wq,wk,wv,wg,wo,wfg,wf2
WS_F1 = 2.0 ** 2        # wf1 (low so f8 = ps1*sig stays < 240)
XS = 2.0 ** 3           # x / h fp8 shadow scale
QS = 2.0 ** 6           # q8/k8 scale (on top of folded QSCALE for q)
VS = 2.0 ** 4           # v8 / ao8 scale

_cached = {}


def _build_nc():
    nc = bacc.Bacc("TRN2", target_bir_lowering=False, debug=False,
                   num_devices=NCORES)

    def din(name, shape, dtype):
        return nc.dram_tensor(name, list(shape), dtype, kind="ExternalInput")

    t = {}
    t["cgmT"] = din("cgmT", [DFEAT, NTOK], BF16)
    t["w_in"] = din("w_in", [DFEAT, D], BF16)
    t["b_in_c"] = din("b_in_c", [128, DC], F32)
    t["b_in8_c"] = din("b_in8_c", [128, DC], F32)
    for w in ("wq8", "wk8", "wv8", "wg8", "wo8"):
        t[w] = din(w, [NL, 128, DC, D], FP8)
    t["wf18"] = din("wf18", [NL, 128, DC, FF], FP8)
    t["wfg8"] = din("wfg8", [NL, 128, DC, FF], FP8)
    t["wf28"] = din("wf28", [NL, 128, FC, D], FP8)
    for bn in ("bq_c", "bk_c", "bg_c", "bo_c", "bf2_c",
               "ln1_s_c", "ln1_ns_c", "ln1_b_c", "ln1_b8_c",
               "ln2_s_c", "ln2_ns_c", "bx_c", "bx8_c"):
        t[bn] = din(bn, [128, NL, DC], F32)
    t["bxp_c"] = din("bxp_c", [128, DC], F32)
    t["bf1_c"] = din("bf1_c", [128, NL, FC], F32)
    t["bfg_c"] = din("bfg_c", [128, NL, FC], F32)
    t["bv_bc"] = din("bv_bc", [128, NL, D], FP8)
    t["exptab"] = din("exptab", [128, NL, 143], BF16)
    t["wd1"] = din("wd1", [128, 17, 128], BF16)
    t["bd1_c"] = din("bd1_c", [128, 1], F32)
    t["ln3_s_c"] = din("ln3_s_c", [128, 1], F32)
    t["ln3_ns_c"] = din("ln3_ns_c", [128, 1], F32)
    t["ln3_b_c"] = din("ln3_b_c", [128, 1], F32)
    t["wd2"] = din("wd2", [128, 128], BF16)
    t["bd2_c"] = din("bd2_c", [128, 1], F32)
    t["wout"] = din("wout", [128, 1], BF16)
    t["bout_t"] = din("bout_t", [1, 1], F32)
    t["otherT"] = din("otherT", [128, BPC], BF16)
    y_out = nc.dram_tensor("y", [1, BPC], F32, kind="ExternalOutput")

    with tile.TileContext(nc, pool_alloc_mode="queue") as tc:
        _emit(nc, tc, t, y_out)
    nc.compile()
    return nc


class Ctx:
    pass


def _emit(nc, tc, t, y_out):
    with (
        tc.tile_pool(name="persist", bufs=1) as pp,
        tc.tile_pool(name="xq", bufs=4) as xqp,
        tc.tile_pool(name="mm_psum", bufs=4, space="PSUM") as mmp,
        tc.tile_pool(name="at_psum", bufs=2, space="PSUM") as app,
        tc.tile_pool(name="st_psum", bufs=1, space="PSUM") as stp,
        tc.tile_pool(name="lnp", bufs=2) as lnp,
        tc.tile_pool(name="resp", bufs=1) as rp,
        tc.tile_pool(name="sqp", bufs=2) as sqp,
    ):
        c = Ctx()
        c.t = t
        c.mmp, c.app, c.stp = mmp, app, stp
        c.lnp, c.rp, c.sqp, c.xqp = lnp, rp, sqp, xqp
        c.pool_pending = []

        # persistent state
        c.xT = pp.tile([128, DC, NTOK], BF16, name="xT")
        c.h = pp.tile([128, DC, NTOK], BF16, name="hT")
        c.h8 = pp.tile([128, DC, NTOK], FP8, name="h8T")
        c.ones_col_bf = pp.tile([128, 1], BF16, name="ones_col")
        nc.vector.memset(c.ones_col_bf, 1.0)
        c.ones128 = pp.tile([128, 128], BF16, name="ones128")
        nc.vector.memset(c.ones128, 1.0)
        c.ones128r = pp.tile([128, 128], mybir.dt.float32r, name="ones128r")
        nc.vector.memset(c.ones128r.bitcast(F32), 1.0)
        c.eps_col = pp.tile([128, 1], F32, name="eps_col")
        nc.vector.memset(c.eps_col, EPS)

        # small constants
        consts = {}
        for name in ("b_in_c", "b_in8_c", "bq_c", "bk_c", "bg_c", "bo_c",
                     "bf2_c", "ln1_s_c", "ln1_ns_c", "ln1_b_c", "ln1_b8_c",
                     "ln2_s_c", "ln2_ns_c", "bx_c", "bx8_c", "bxp_c",
                     "bf1_c", "bfg_c",
                     "bv_bc", "exptab", "bd1_c", "ln3_s_c", "ln3_ns_c",
                     "ln3_b_c", "wd2", "bd2_c", "wout", "bout_t", "otherT"):
            ap = t[name]
            tl = pp.tile(list(ap.shape), ap.dtype, name=f"c_{name}")
            nc.gpsimd.dma_start(out=tl[:], in_=ap[:])
            consts[name] = tl
        c.consts = consts

        with tc.tile_pool(name="wlayer", bufs=1) as wp:
            c.wp = wp
            # layer-0 big weights start streaming before the input proj
            w5 = _load_layer_weights(nc, c, 0)

            # ---- input projection: xT / xq8 ----
            c.xq = [None] * NG
            with tc.tile_pool(name="inp", bufs=1) as inp:
                cgmT_s = inp.tile([DFEAT, NTOK], BF16, name="cgm_s")
                nc.sync.dma_start(out=cgmT_s[:], in_=t["cgmT"][:])
                w_in_s = inp.tile([DFEAT, D], BF16, name="w_in_s")
                nc.sync.dma_start(out=w_in_s[:], in_=t["w_in"][:])
                for g in range(NG):
                    tok = slice(g * GT, (g + 1) * GT)
                    xq_g = xqp.tile([128, DC, GT], FP8, tag="xq", name="xq_g")
                    for dd in range(DC):
                        ps = mmp.tile([128, GT], F32, tag="mm", name="ps_in")
                        nc.tensor.matmul(
                            ps[:], w_in_s[:, dd * 128:(dd + 1) * 128],
                            cgmT_s[:, tok], start=True, stop=True)
                        nc.vector.tensor_scalar_add(
                            out=c.xT[:, dd, tok], in0=ps[:],
                            scalar1=consts["b_in_c"][:, dd:dd + 1])
                        nc.scalar.activation(
                            out=xq_g[:, dd, :], in_=ps[:], func=AF.Identity,
                            bias=consts["b_in8_c"][:, dd:dd + 1], scale=XS)
                    c.xq[g] = xq_g

            # ---- transformer layers ----
            for i in range(NL):
                w5_next = _layer(nc, tc, c, i, w5)
                w5 = w5_next

        # ---- head ----
        _head(nc, tc, c, y_out)


def _load_layer_weights(nc, c, i):
    """Preload the V weight whole-layer; Q/K/O/G stream in per-chunk."""
    w5 = {}
    for nm in ("wv8",):
        tl = c.wp.tile([128, DC, D], FP8, tag=nm, name=f"{nm}_s")
        nc.sync.dma_start(out=tl[:], in_=c.t[nm][i])
        w5[nm] = tl
    return w5


_CHUNK_BUFS = {"wq8": 6, "wk8": 6, "wg8": 4, "wo8": 4}


def _w_chunk(nc, c, nm, i, dd, queue="sync"):
    ch = c.wp.tile([128, DC, 128], FP8, tag=f"{nm}_ch", bufs=_CHUNK_BUFS[nm],
                   name=f"{nm}_ch")
    eng = nc.sync if queue == "sync" else nc.gpsimd
    eng.dma_start(out=ch[:], in_=c.t[nm][i, :, :, dd * 128:(dd + 1) * 128])
    return ch


def _layer(nc, tc, c, i, w5):
    with (
        tc.tile_pool(name="grp", bufs=2) as gp,
        tc.tile_pool(name="attw", bufs=2) as at,
    ):
        c.gp, c.at = gp, at
        qkv = [None, None]   # rotating (q8, k8, v8) per group parity
        qkv[0] = _emit_qkv(nc, c, i, w5, 0, fillers=None)

        sig_t = None
        for g in range(NG):
            fillers = []
            if g + 1 < NG:
                qkv[(g + 1) % 2] = _emit_qkv(nc, c, i, w5, g + 1,
                                             fillers=fillers)
            else:
                # last group: the gate projection (which only needs xq)
                # becomes the filler material
                sig_t = _emit_gate(nc, c, i, w5, g, fillers=fillers)
            ao8 = _attention(nc, c, i, g, qkv[g % 2], fillers)
            _og_ln1(nc, c, i, w5, g, ao8, qkv[g % 2],
                    sig_pre=sig_t if g == NG - 1 else None)

    # layer i+1 weights stream during this layer's FFN
    w5_next = _load_layer_weights(nc, c, i + 1) if i + 1 < NL else None

    _ffn(nc, tc, c, i)
    return w5_next


def _emit_qkv(nc, c, i, w5, g, fillers):
    """Q/K/V projections for group g. If fillers is a list, append one
    closure per PSUM-group instead of emitting directly."""
    consts = c.consts
    tok = slice(g * GT, (g + 1) * GT)
    q8 = c.gp.tile([128, H, GT], FP8, tag="q8", name="q8")
    k8 = c.gp.tile([128, H, GT], FP8, tag="k8", name="k8")
    v8 = c.gp.tile([128, GB, D], FP8, tag="v8", name="v8")
    xq = c.xq[g]

    def q_chunk(dd):
        def f():
            wq_ch = _w_chunk(nc, c, "wq8", i, dd)
            ps = c.mmp.tile([128, GT], F32, tag="mm", name="psq")
            for kc in range(DC2):
                nc.tensor.matmul(
                    ps[:], wq_ch[:, 2 * kc:2 * kc + 2, :],
                    xq[:, 2 * kc:2 * kc + 2, :],
                    start=(kc == 0), stop=(kc == DC2 - 1), perf_mode=DR)
            nc.scalar.activation(
                out=q8[:, dd, :], in_=ps[:], func=AF.Identity,
                bias=consts["bq_c"][:, i, dd:dd + 1],
                scale=QS * QSCALE / (WS * XS))
        return f

    def k_chunk(dd):
        def f():
            wk_ch = _w_chunk(nc, c, "wk8", i, dd)
            ps = c.mmp.tile([128, GT], F32, tag="mm", name="psk")
            for kc in range(DC2):
                nc.tensor.matmul(
                    ps[:], wk_ch[:, 2 * kc:2 * kc + 2, :],
                    xq[:, 2 * kc:2 * kc + 2, :],
                    start=(kc == 0), stop=(kc == DC2 - 1), perf_mode=DR)
            nc.scalar.activation(
                out=k8[:, dd, :], in_=ps[:], func=AF.Identity,
                bias=consts["bk_c"][:, i, dd:dd + 1],
                scale=QS / (WS * XS))
        return f

    def v_chunk(cc, jj):
        def f():
            ps = c.mmp.tile([128, 512], F32, tag="mm", name="psv")
            for kc in range(DC2):
                nc.tensor.matmul(
                    ps[:], xq[:, 2 * kc:2 * kc + 2, jj * L:(jj + 1) * L],
                    w5["wv8"][:, 2 * kc:2 * kc + 2,
                              cc * 512:(cc + 1) * 512],
                    start=(kc == 0), stop=(kc == DC2 - 1), perf_mode=DR)
            nc.vector.scalar_tensor_tensor(
                out=v8[:, jj, cc * 512:(cc + 1) * 512], in0=ps[:],
                scalar=VS / (WS * XS),
                in1=consts["bv_bc"][:, i, cc * 512:(cc + 1) * 512],
                op0=OP.mult, op1=OP.add)
        return f

    closures = ([q_chunk(dd) for dd in range(DC)]
                + [k_chunk(dd) for dd in range(DC)]
                + [v_chunk(cc, jj) for cc in range(2) for jj in range(GB)])
    if fillers is None:
        for f in closures:
            f()
    else:
        fillers.extend(closures)
    return q8, k8, v8


def _emit_gate(nc, c, i, w5, g, fillers):
    """Gate projection sigmoid(x@Wg+bg) for group g (filler closures)."""
    consts = c.consts
    xq = c.xq[g]
    sig_t = c.rp.tile([128, DC, GT], BF16, tag="sig", name="sig_g")

    def g_chunk(dd):
        def f():
            wg_ch = _w_chunk(nc, c, "wg8", i, dd, queue="gpsimd")
            ps = c.mmp.tile([128, GT], F32, tag="mm", name="psg")
            for kc in range(DC2):
                nc.tensor.matmul(
                    ps[:], wg_ch[:, 2 * kc:2 * kc + 2, :],
                    xq[:, 2 * kc:2 * kc + 2, :],
                    start=(kc == 0), stop=(kc == DC2 - 1), perf_mode=DR)
            nc.scalar.activation(
                out=sig_t[:, dd, :], in_=ps[:], func=AF.Sigmoid,
                bias=consts["bg_c"][:, i, dd:dd + 1], scale=1.0 / (WS * XS))
        return f

    fillers.extend(g_chunk(dd) for dd in range(DC))
    return sig_t


def _attention(nc, c, i, g, qkv, fillers):
    """Attention for group g, one wave per batch (8 heads in 2 half-waves).
    Emits filler closures between dependency steps to keep the PE dense."""
    q8, k8, v8 = qkv
    fi = iter(fillers)

    def pump(n):
        for _ in range(n):
            f = next(fi, None)
            if f is None:
                return
            f()

    ao8 = c.gp.tile([128, H, GB, L], FP8, tag="ao8", bufs=1, name="ao8")
    for jj in range(GB):
        b_local = g * GB + jj
        jtok = slice(jj * L, (jj + 1) * L)
        etab = c.consts["exptab"][:, i, 15 - b_local:143 - b_local]
        pa = [None, None]
        awe2 = [None, None]
        for half in range(2):
            h0 = half * 4
            pa[half] = c.app.tile([128, 512], F32, tag="pa", name="pa")
            for hh in range(4):
                nc.tensor.matmul(
                    pa[half][:, hh * L:(hh + 1) * L],
                    k8[:, h0 + hh, jtok], q8[:, h0 + hh, jtok],
                    start=True, stop=True)
        pump(2)
        rb = [None, None]
        for half in range(2):
            awe = c.at.tile([128, 512], BF16, tag="awe", name="awe")
            nc.scalar.activation(out=awe[:], in_=pa[half][:], func=AF.Exp,
                                 scale=1.0 / (QS * QS))
            # in-place multiply by the per-batch exp(bias) table
            nc.vector.tensor_tensor(
                out=awe[:].rearrange("p (h q) -> p h q", q=L),
                in0=awe[:].rearrange("p (h q) -> p h q", q=L),
                in1=etab.unsqueeze(1).to_broadcast([128, 4, L]), op=OP.mult)
            awe2[half] = awe
        pump(2)
        for half in range(2):
            # softmax sums land in partition 0 of the (already-read) logit
            # psum tile — saves a PSUM bank
            sm = pa[half][0:1, :]
            nc.tensor.matmul(sm, c.ones_col_bf[:, 0:1], awe2[half][:],
                             start=True, stop=True)
            smr = c.at.tile([1, 512], F32, tag="smr", bufs=1, name="smr")
            nc.scalar.copy(out=smr[:], in_=sm)
            sb = c.lnp.tile([128, 512], F32, tag="rs", bufs=3, name="sb")
            nc.gpsimd.partition_broadcast(sb[:], smr[:])
            rb[half] = c.lnp.tile([128, 512], F32, tag="rs", bufs=3, name="rb")
            nc.vector.reciprocal_approx_fast(out=rb[half][:], in_=sb[:])
        pump(2)
        for half in range(2):
            h0 = half * 4
            pao = c.app.tile([128, 512], F32, tag="pa", name="pao")
            for hh in range(4):
                nc.tensor.matmul(
                    pao[:, hh * L:(hh + 1) * L],
                    v8[:, jj, (h0 + hh) * DK:(h0 + hh + 1) * DK],
                    awe2[half][:, hh * L:(hh + 1) * L], start=True, stop=True)
            nc.vector.tensor_tensor(
                out=ao8[:, h0:h0 + 4, jj, :],
                in0=pao[:].rearrange("p (h q) -> p h q", q=L),
                in1=rb[half][:].rearrange("p (h q) -> p h q", q=L),
                op=OP.mult)
    # drain any remaining fillers
    pump(1 << 30)
    return ao8


def _og_ln1(nc, c, i, w5, g, ao8, qkv, sig_pre):
    """Gate + O-projection + residual + LN1 for group g."""
    consts = c.consts
    tok = slice(g * GT, (g + 1) * GT)
    xq = c.xq[g]

    # gate sigmoid (unless pre-emitted as attention fillers for the last
    # group); emitting it here keeps the scalar queue clear of sigmoids
    # during the attention waves and the psum rotation unclogged
    if sig_pre is None:
        sig_t = c.rp.tile([128, DC, GT], BF16, tag="sig", name="sig_g")
        for dd in range(DC):
            wg_ch = _w_chunk(nc, c, "wg8", i, dd, queue="gpsimd")
            ps = c.mmp.tile([128, GT], F32, tag="mm", name="psg")
            for kc in range(DC2):
                nc.tensor.matmul(
                    ps[:], wg_ch[:, 2 * kc:2 * kc + 2, :],
                    xq[:, 2 * kc:2 * kc + 2, :],
                    start=(kc == 0), stop=(kc == DC2 - 1), perf_mode=DR)
            nc.scalar.activation(
                out=sig_t[:, dd, :], in_=ps[:], func=AF.Sigmoid,
                bias=consts["bg_c"][:, i, dd:dd + 1], scale=1.0 / (WS * XS))
    else:
        sig_t = sig_pre

    res_t = c.rp.tile([128, DC, GT], BF16, tag="res", name="res_t")
    ps_s = c.stp.tile([128, GT], F32, tag="ss", name="ps_s")
    ps_q = c.stp.tile([128, GT], F32, tag="sq", name="ps_q")
    for dd in range(DC):
        wo_ch = _w_chunk(nc, c, "wo8", i, dd, queue="gpsimd")
        pso = c.mmp.tile([128, GT], F32, tag="mm", name="pso")
        for kc in range(DC2):
            nc.tensor.matmul(
                pso[:], wo_ch[:, 2 * kc:2 * kc + 2, :],
                ao8[:, 2 * kc:2 * kc + 2, :, :],
                start=(kc == 0), stop=(kc == DC2 - 1), perf_mode=DR)
        t1 = c.sqp.tile([128, GT], F32, tag="t1", bufs=1, name="t1")
        nc.scalar.activation(
            out=t1[:], in_=pso[:], func=AF.Identity,
            bias=consts["bo_c"][:, i, dd:dd + 1], scale=1.0 / (WS * VS))
        nc.vector.tensor_mul(out=t1[:], in0=t1[:], in1=sig_t[:, dd, :])
        # res = (xT_nob + bx) + gated-attn; bx folds the previous layer's
        # deferred LN2 bias (zero for layer 0, whose xT carries b_in)
        nc.vector.scalar_tensor_tensor(
            out=res_t[:, dd, :], in0=c.xT[:, dd, tok],
            scalar=consts["bx_c"][:, i, dd:dd + 1], in1=t1[:],
            op0=OP.add, op1=OP.add)
        sq = c.sqp.tile([128, GT], BF16, tag="sq", name="sq")
        nc.scalar.activation(out=sq[:], in_=res_t[:, dd, :], func=AF.Square)
        nc.tensor.matmul(ps_s[:], c.ones128[:], res_t[:, dd, :],
                         start=(dd == 0), stop=(dd == DC - 1))
        nc.tensor.matmul(ps_q[:], c.ones128[:], sq[:],
                         start=(dd == 0), stop=(dd == DC - 1))

    rs_f, mrs_f = _ln_rows_full(nc, c, ps_s, ps_q, GT, 1.0 / D)
    for dd in range(DC):
        u = c.lnp.tile([128, GT], BF16, tag="u", bufs=1, name="u")
        nc.vector.scalar_tensor_tensor(
            out=u[:], in0=res_t[:, dd, :],
            scalar=consts["ln1_s_c"][:, i, dd:dd + 1], in1=rs_f[:],
            op0=OP.mult, op1=OP.mult)
        # h is stored WITHOUT ln1_b; the bias is folded into the two
        # consumers (h8 shadow below, f2-residual add in _ffn)
        nc.vector.scalar_tensor_tensor(
            out=c.h[:, dd, tok], in0=mrs_f[:],
            scalar=consts["ln1_ns_c"][:, i, dd:dd + 1], in1=u[:],
            op0=OP.mult, op1=OP.add)
        nc.scalar.activation(out=c.h8[:, dd, tok], in_=c.h[:, dd, tok],
                             func=AF.Identity,
                             bias=consts["ln1_b8_c"][:, i, dd:dd + 1],
                             scale=XS)


def _ln_rows_full(nc, c, ps_s, ps_q, n, inv_d):
    """Full-width LN stats: rs = 1/sqrt(var+eps), mrs = mean*rs as
    [128, n] tiles (all rows identical)."""
    m_f = c.lnp.tile([128, 512], F32, tag="m", bufs=1, name="m_f")[:, :n]
    nc.vector.tensor_scalar_mul(out=m_f, in0=ps_s[:], scalar1=inv_d)
    m2 = c.lnp.tile([128, 512], F32, tag="tmp", bufs=2, name="m2")[:, :n]
    nc.vector.tensor_mul(out=m2, in0=m_f, in1=m_f)
    var = c.lnp.tile([128, 512], F32, tag="tmp", bufs=2, name="var")[:, :n]
    nc.vector.scalar_tensor_tensor(out=var, in0=ps_q[:], scalar=inv_d,
                                   in1=m2, op0=OP.mult, op1=OP.subtract)
    std = c.lnp.tile([128, 512], F32, tag="tmp", bufs=2, name="std")[:, :n]
    nc.scalar.activation(out=std, in_=var, func=AF.Sqrt, bias=c.eps_col[:],
                         scale=1.0)
    rs_f = c.lnp.tile([128, 512], F32, tag="rs", bufs=3, name="rs_f")[:, :n]
    nc.vector.reciprocal_approx_fast(out=rs_f, in_=std)
    # bf16 copies let the apply STTs hit the DVE 2x mode
    rs_b = c.lnp.tile([128, 512], BF16, tag="rsb", name="rs_b")[:, :n]
    nc.scalar.copy(out=rs_b, in_=rs_f)
    mrs_b = c.lnp.tile([128, 512], BF16, tag="mrsb", name="mrs_b")[:, :n]
    nc.vector.tensor_mul(out=mrs_b, in0=m_f, in1=rs_f)
    return rs_b, mrs_b


def _ffn(nc, tc, c, i):
    consts = c.consts
    with (
        tc.tile_pool(name="fbuf", bufs=1) as fp,
        tc.tile_pool(name="fwch", bufs=4) as wc,
        tc.tile_pool(name="fw2ch", bufs=2) as wc2,
    ):
        for sub in range(NG):
            tok = slice(sub * GT, (sub + 1) * GT)
            f8 = fp.tile([128, FC, GT], FP8, tag="f8", name="f8")
            # --- f8 = (h@Wf1 + bf1) * sigmoid(h@Wfg + bfg), fp8-scaled ---
            for fc in range(FC):
                if fc % 8 == 4 and c.pool_pending:
                    c.pool_pending.pop(0)()
                wf1_ch = wc.tile([128, DC, 128], FP8, tag="wf1", bufs=4, name="wf1c")
                nc.sync.dma_start(
                    out=wf1_ch[:],
                    in_=c.t["wf18"][i, :, :, fc * 128:(fc + 1) * 128])
                wfg_ch = wc.tile([128, DC, 128], FP8, tag="wfg", bufs=4, name="wfgc")
                nc.gpsimd.dma_start(
                    out=wfg_ch[:],
                    in_=c.t["wfg8"][i, :, :, fc * 128:(fc + 1) * 128])
                ps1 = c.mmp.tile([128, GT], F32, tag="mm", name="ps1")
                psg = c.mmp.tile([128, GT], F32, tag="mm", name="psfg")
                for kc in range(DC2):
                    nc.tensor.matmul(
                        ps1[:], wf1_ch[:, 2 * kc:2 * kc + 2, :],
                        c.h8[:, 2 * kc:2 * kc + 2, tok],
                        start=(kc == 0), stop=(kc == DC2 - 1), perf_mode=DR)
                for kc in range(DC2):
                    nc.tensor.matmul(
                        psg[:], wfg_ch[:, 2 * kc:2 * kc + 2, :],
                        c.h8[:, 2 * kc:2 * kc + 2, tok],
                        start=(kc == 0), stop=(kc == DC2 - 1), perf_mode=DR)
                sig = c.sqp.tile([128, GT], BF16, tag="fsig", name="fsig")
                nc.scalar.activation(
                    out=sig[:], in_=psg[:], func=AF.Sigmoid,
                    bias=consts["bfg_c"][:, i, fc:fc + 1],
                    scale=1.0 / (WS * XS))
                nc.vector.scalar_tensor_tensor(
                    out=f8[:, fc, :], in0=ps1[:],
                    scalar=consts["bf1_c"][:, i, fc:fc + 1], in1=sig[:],
                    op0=OP.add, op1=OP.mult)

            # --- f @ Wf2 + bf2 + h, LN2, stochastic-depth blend into xT ---
            res_t = c.rp.tile([128, DC, GT], BF16, tag="res", name="res2")
            ps_s = c.stp.tile([128, GT], F32, tag="ss", name="ps_s2")
            ps_q = c.stp.tile([128, GT], F32, tag="sq", name="ps_q2")
            for dd in range(DC):
                wf2_ch = wc2.tile([128, FC, 128], FP8, tag="wf2", name="wf2c")
                nc.gpsimd.dma_start(
                    out=wf2_ch[:],
                    in_=c.t["wf28"][i, :, :, dd * 128:(dd + 1) * 128])
                ps2 = c.mmp.tile([128, GT], F32, tag="mm", name="ps2")
                for fc in range(FC2):
                    nc.tensor.matmul(
                        ps2[:], wf2_ch[:, 2 * fc:2 * fc + 2, :],
                        f8[:, 2 * fc:2 * fc + 2, :],
                        start=(fc == 0), stop=(fc == FC2 - 1), perf_mode=DR)
                t1 = c.sqp.tile([128, GT], F32, tag="t1", bufs=1, name="ft1")
                nc.scalar.activation(
                    out=t1[:], in_=ps2[:], func=AF.Identity,
                    bias=consts["bf2_c"][:, i, dd:dd + 1],
                    scale=1.0 / (WS * WS_F1 * XS))
                # h is stored without ln1_b; add it back here
                nc.vector.scalar_tensor_tensor(
                    out=res_t[:, dd, :], in0=c.h[:, dd, tok],
                    scalar=consts["ln1_b_c"][:, i, dd:dd + 1], in1=t1[:],
                    op0=OP.add, op1=OP.add)
                sq = c.sqp.tile([128, GT], BF16, tag="sq", name="fsq")
                nc.scalar.activation(out=sq[:], in_=res_t[:, dd, :],
                                     func=AF.Square)
                nc.tensor.matmul(ps_s[:], c.ones128[:], res_t[:, dd, :],
                                 start=(dd == 0), stop=(dd == DC - 1))
                nc.tensor.matmul(ps_q[:], c.ones128[:], sq[:],
                                 start=(dd == 0), stop=(dd == DC - 1))

            rs_f, mrs_f = _ln_rows_full(nc, c, ps_s, ps_q, GT, 1.0 / D)
            sv = SURVIVE[i]
            for dd in range(DC):
                u = c.lnp.tile([128, GT], BF16, tag="u", bufs=1, name="fu")
                nc.vector.scalar_tensor_tensor(
                    out=u[:], in0=res_t[:, dd, :],
                    scalar=consts["ln2_s_c"][:, i, dd:dd + 1], in1=rs_f[:],
                    op0=OP.mult, op1=OP.mult)
                if sv == 1.0:
                    nc.vector.scalar_tensor_tensor(
                        out=c.xT[:, dd, tok], in0=mrs_f[:],
                        scalar=consts["ln2_ns_c"][:, i, dd:dd + 1], in1=u[:],
                        op0=OP.mult, op1=OP.add)
                else:
                    v1 = c.lnp.tile([128, GT], BF16, tag="v1", bufs=1,
                                    name="fv1")
                    nc.vector.scalar_tensor_tensor(
                        out=v1[:], in0=mrs_f[:],
                        scalar=consts["ln2_ns_c"][:, i, dd:dd + 1], in1=u[:],
                        op0=OP.mult, op1=OP.add)
                    nc.vector.scalar_tensor_tensor(
                        out=c.xT[:, dd, tok], in0=c.xT[:, dd, tok],
                        scalar=1.0 - sv, in1=v1[:], op0=OP.mult, op1=OP.add)
                # xT is stored WITHOUT the (survive-scaled) ln2 bias; it is
                # folded into the consumers (xq shadow, next layer's
                # O-residual via bx_c, pooling via bxp_c)
                if i + 1 < NL:
                    # fp8 shadow for the next layer's projections
                    if dd == 0:
                        c.xq[sub] = c.xqp.tile([128, DC, GT], FP8, tag="xq",
                                               name="xq_n")
                    nc.scalar.activation(out=c.xq[sub][:, dd, :],
                                         in_=c.xT[:, dd, tok],
                                         func=AF.Identity,
                                         bias=consts["bx8_c"][:, i + 1,
                                                             dd:dd + 1],
                                         scale=XS)
            if i + 1 == NL:
                # head pooling: enqueue one small reduce per batch; they are
                # drained inside the next sub's fc loop so the 1.2us vector
                # ops never block the PE's psum drains in a burst
                if sub == 0:
                    c.pool_sum = c.lnp.tile([128, DC, BPC], F32, tag="pls",
                                            bufs=1, name="pool_sum")
                    c.pool_max = c.lnp.tile([128, DC, BPC], F32, tag="plm",
                                            bufs=1, name="pool_max")

                def mk_pool(b_abs):
                    def f():
                        xv = c.xT[:, :, b_abs * L:(b_abs + 1) * L]
                        nc.vector.tensor_reduce(
                            out=c.pool_sum[:, :, b_abs], in_=xv,
                            axis=mybir.AxisListType.X, op=OP.add)
                        nc.vector.tensor_reduce(
                            out=c.pool_max[:, :, b_abs], in_=xv,
                            axis=mybir.AxisListType.X, op=OP.max)
                    return f
                for jj in range(GB):
                    c.pool_pending.append(mk_pool(sub * GB + jj))


def _head(nc, tc, c, y_out):
    consts = c.consts
    with tc.tile_pool(name="head", bufs=1) as hp:
        for f in c.pool_pending:
            f()
        c.pool_pending = []
        wd1_s = hp.tile([128, 17, 128], BF16, name="wd1_s")
        nc.sync.dma_start(out=wd1_s[:], in_=c.t["wd1"][:])
        poolT = hp.tile([128, 17, BPC], BF16, name="poolT")
        for dd in range(DC):
            # mean/max pooling of xT_nob + deferred final LN2 bias (bxp)
            nc.vector.tensor_scalar(
                out=poolT[:, dd, :], in0=c.pool_sum[:, dd, :],
                scalar1=1.0 / L, scalar2=consts["bxp_c"][:, dd:dd + 1],
                op0=OP.mult, op1=OP.add)
            nc.vector.tensor_scalar_add(
                out=poolT[:, DC + dd, :], in0=c.pool_max[:, dd, :],
                scalar1=consts["bxp_c"][:, dd:dd + 1])
        nc.gpsimd.tensor_copy(out=poolT[:, 16, :], in_=consts["otherT"][:])

        # y1 = relu(pooled @ Wd1 + bd1)   [128 dout, 16]
        ps1 = c.mmp.tile([128, GT], F32, tag="mm", name="hps")[:, 0:BPC]
        for cc in range(17):
            nc.tensor.matmul(ps1, wd1_s[:, cc, :], poolT[:, cc, :],
                             start=(cc == 0), stop=(cc == 16))
        y1 = hp.tile([128, BPC], mybir.dt.float32r, name="y1")
        with nc.allow_low_precision(reason="fp32r head LN within tolerance"):
            nc.scalar.activation(out=y1[:], in_=ps1, func=AF.Relu,
                                 bias=consts["bd1_c"][:], scale=1.0)

        # LN3 over the 128 features (partition dim), full-width stats
        sq3 = hp.tile([128, BPC], mybir.dt.float32r, name="sq3")
        with nc.allow_low_precision(reason="fp32r head LN within tolerance"):
            nc.vector.tensor_mul(out=sq3[:], in0=y1[:].bitcast(F32),
                                 in1=y1[:].bitcast(F32))
        ps_s = c.stp.tile([128, GT], F32, tag="ss", name="hs")[:, 0:BPC]
        ps_q = c.stp.tile([128, GT], F32, tag="sq", name="hq")[:, 0:BPC]
        nc.tensor.matmul(ps_s, c.ones128r[:], y1[:], start=True, stop=True)
        nc.tensor.matmul(ps_q, c.ones128r[:], sq3[:], start=True, stop=True)
        rs_f, mrs_f = _ln_rows_full(nc, c, ps_s, ps_q, BPC, 1.0 / 128)
        u3 = hp.tile([128, BPC], F32, name="u3")
        nc.vector.scalar_tensor_tensor(
            out=u3[:], in0=y1[:].bitcast(F32), scalar=consts["ln3_s_c"][:],
            in1=rs_f[:], op0=OP.mult, op1=OP.mult)
        yln0 = hp.tile([128, BPC], F32, name="yln0")
        nc.vector.scalar_tensor_tensor(
            out=yln0[:], in0=mrs_f[:], scalar=consts["ln3_ns_c"][:],
            in1=u3[:], op0=OP.mult, op1=OP.add)
        yln = hp.tile([128, BPC], BF16, name="yln")
        nc.vector.tensor_scalar_add(out=yln[:], in0=yln0[:],
                                    scalar1=consts["ln3_b_c"][:])

        # y2 = relu(yln @ Wd2 + bd2); y = y2 @ Wout + bout
        ps2 = c.mmp.tile([128, GT], F32, tag="mm", name="hps2")[:, 0:BPC]
        nc.tensor.matmul(ps2, consts["wd2"][:], yln[:], start=True, stop=True)
        y2 = hp.tile([128, BPC], BF16, name="y2")
        nc.scalar.activation(out=y2[:], in_=ps2, func=AF.Relu,
                             bias=consts["bd2_c"][:], scale=1.0)
        psy = c.mmp.tile([128, GT], F32, tag="mm", name="hpsy")[0:1, 0:BPC]
        nc.tensor.matmul(psy, consts["wout"][:], y2[:], start=True, stop=True)
        yfin = hp.tile([1, BPC], F32, name="yfin")
        nc.vector.tensor_tensor(
            out=yfin[:], in0=psy,
            in1=consts["bout_t"][:].to_broadcast([1, BPC]), op=OP.add)
        nc.sync.dma_start(out=y_out[:], in_=yfin[:])


# ---------------------------------------------------------------------------
# host side
# ---------------------------------------------------------------------------

def _bf(x):
    return np.ascontiguousarray(np.asarray(x, np.float32)).astype(
        ml_dtypes.bfloat16)


def _f32(x):
    return np.ascontiguousarray(np.asarray(x, np.float32))


def _fp8(x, scale):
    a = np.asarray(x, np.float32) * np.float32(scale)
    a = np.clip(a, -240.0, 240.0)
    return np.ascontiguousarray(a).astype(ml_dtypes.float8_e4m3)


def _prep_shared(I):
    sv = np.array(SURVIVE, np.float32)
    s = {}
    s["w_in"] = _bf(I["W_in"])
    b_in = np.asarray(I["b_in"], np.float32).reshape(DC, 128).T
    s["b_in_c"] = _f32(b_in)
    s["b_in8_c"] = _f32(b_in * XS)
    for nm, W, ws in (("wq8", I["Wq"], WS), ("wk8", I["Wk"], WS),
                      ("wv8", I["Wv"], WS), ("wg8", I["Wg"], WS),
                      ("wo8", I["Wo"], WS)):
        s[nm] = _fp8(np.asarray(W, np.float32).reshape(NL, DC, 128, D)
                     .transpose(0, 2, 1, 3), ws)
    s["wf18"] = _fp8(np.asarray(I["Wf1"], np.float32)
                     .reshape(NL, DC, 128, FF).transpose(0, 2, 1, 3), WS_F1)
    s["wfg8"] = _fp8(np.asarray(I["Wfg"], np.float32)
                     .reshape(NL, DC, 128, FF).transpose(0, 2, 1, 3), WS)
    s["wf28"] = _fp8(np.asarray(I["Wf2"], np.float32)
                     .reshape(NL, FC, 128, D).transpose(0, 2, 1, 3), WS)

    def col(b, nch):
        return _f32(np.asarray(b, np.float32).reshape(NL, nch, 128)
                    .transpose(2, 0, 1))

    s["bq_c"] = col(np.asarray(I["bq"], np.float32) * (QS * QSCALE), DC)
    s["bk_c"] = col(np.asarray(I["bk"], np.float32) * QS, DC)
    s["bg_c"] = col(I["bg"], DC)
    s["bo_c"] = col(I["bo"], DC)
    s["bf2_c"] = col(I["bf2"], DC)
    s["bf1_c"] = col(np.asarray(I["bf1"], np.float32) * (WS_F1 * XS), FC)
    s["bfg_c"] = col(I["bfg"], FC)
    ln1_s = np.asarray(I["ln1_s"], np.float32)
    ln2_s = np.asarray(I["ln2_s"], np.float32) * sv[:, None]
    s["ln1_s_c"] = col(ln1_s, DC)
    s["ln1_ns_c"] = col(-ln1_s, DC)
    s["ln1_b_c"] = col(I["ln1_b"], DC)
    s["ln1_b8_c"] = col(np.asarray(I["ln1_b"], np.float32) * XS, DC)
    s["ln2_s_c"] = col(ln2_s, DC)
    s["ln2_ns_c"] = col(-ln2_s, DC)
    # deferred (survive-scaled) LN2 bias for the residual stream:
    # bx[i] = bias carried by xT entering layer i (0 for layer 0 — its xT
    # includes b_in directly); bxp = bias at pooling time.
    ln2_b = np.asarray(I["ln2_b"], np.float32)
    bx = np.zeros((NL, D), np.float32)
    running = np.zeros(D, np.float32)
    for li in range(NL):
        bx[li] = running
        running = (1.0 - sv[li]) * running + sv[li] * ln2_b[li]
    s["bx_c"] = col(bx, DC)
    s["bx8_c"] = col(bx * XS, DC)
    s["bxp_c"] = _f32(running.reshape(DC, 128).T)
    s["bv_bc"] = _fp8(np.broadcast_to(
        np.asarray(I["bv"], np.float32)[None] * VS, (128, NL, D)), 1.0)
    s["wd1"] = _bf(np.concatenate(
        [np.asarray(I["Wd1"], np.float32),
         np.zeros((17 * 128 - I["Wd1"].shape[0], 128), np.float32)],
        axis=0).reshape(17, 128, 128).transpose(1, 0, 2))
    s["bd1_c"] = _f32(I["bd1"].reshape(128, 1))
    ln3_s = np.asarray(I["ln3_s"], np.float32)
    s["ln3_s_c"] = _f32(ln3_s.reshape(128, 1))
    s["ln3_ns_c"] = _f32(-ln3_s.reshape(128, 1))
    s["ln3_b_c"] = _f32(I["ln3_b"].reshape(128, 1))
    s["wd2"] = _bf(I["Wd2"])
    s["bd2_c"] = _f32(I["bd2"].reshape(128, 1))
    s["wout"] = _bf(I["Wout"])
    s["bout_t"] = _f32(I["bout"].reshape(1, 1))
    return s


def _prep_core(I, shared, cidx):
    m = dict(shared)
    cgm = np.asarray(I["cgm"], np.float32)
    m["cgmT"] = _bf(cgm[cidx * BPC:(cidx + 1) * BPC].reshape(NTOK, DFEAT).T)
    oth = np.asarray(I["other"], np.float32)[cidx * BPC:(cidx + 1) * BPC]
    m["otherT"] = _bf(np.concatenate(
        [oth.T, np.zeros((128 - OTHER, BPC), np.float32)], axis=0))
    # exp of transposed+flipped rel-pos bias table, per-core column window
    rel = np.asarray(I["rel_emb"], np.float32)          # [NL, 255, 128]
    flippedT = rel[:, ::-1, :].transpose(0, 2, 1)       # [NL, 128, 255]
    lo = 112 - 16 * cidx
    tab = flippedT[:, :, lo:lo + 143]                   # [NL, 128, 143]
    m["exptab"] = _bf(np.exp(tab).transpose(1, 0, 2))   # [128, NL, 143]
    return m


def kernel(**inputs) -> np.ndarray:
    if "nc" not in _cached:
        _cached["nc"] = _build_nc()
    nc = _cached["nc"]
    shared = _prep_shared(inputs)
    in_maps = [_prep_core(inputs, shared, cidx) for cidx in range(NCORES)]
    res = run_bass_kernel_spmd(nc, in_maps, core_ids=list(range(NCORES)))
    y = np.concatenate([res.results[cidx]["y"].reshape(BPC)
                        for cidx in range(NCORES)])
    return y.reshape(B, 1).astype(np.float32)
